# revision 1
# baseline (speedup 1.0000x reference)
"""Distance-weighted self-attention on 8 Trainium2 NeuronCores.

The reference network is rank-1 in the d_model dimension:
  q = h*Wq, k = h*Wk, v = h*Wv  (h = heights column of the input)
so  logits[s,t] = c*h_s*h_t - 0.5*|sz_s - sz_t|   with c = (Wq.Wk)/sqrt(256)
and out[s,:]   = (sum_t softmax(logits)[s,t]*h_t) * Wv.

Each core handles one batch element (B=8). Per core, for each 128-row block
of keys t (partitions) against all 2048 queries s (free dim):
  L  = h_s_rep * (c*h_t[p]) - 0.5*|sig_s_rep - sig_t[p]|   (one fused DVE op)
  E  = exp(L)                                              (scalar engine)
  num/den via PE: lhsT=[h_t|1] stationary, rhs=E in 512-wide slices,
  accumulated over key chunks into PSUM rows [2, 2048].
Then num/den are transposed on-chip to [128, 32] (16 small PE matmuls against
a 2x2 identity accumulating into a zeroed PSUM bank), a = num/den, and
out chunks = a[p] * Wv_rep (outer products split across DVE and ACT), with
the 2MB result DMAed out in four query-quarter chunks on the two HWDGE
queues. The last key chunk is processed in four 512-wide query quarters so
this whole tail pipelines per quarter.

Max-subtraction in softmax is unnecessary: |logits| <= ~12 and the common
factor cancels exactly in num/den.
"""

import os
import sys

import numpy as np

for _p in ("/opt/trn_rl_repo", "/root/.axon_site/_ro/trn_rl_repo"):
    if os.path.isdir(_p) and _p not in sys.path:
        sys.path.append(_p)

import concourse.bacc as bacc
import concourse.bass as bass
import concourse.mybir as mybir
import concourse.tile as tile
from concourse.bass_utils import run_bass_kernel_spmd
from concourse.dve_ops import (
    CUSTOM_DVE_SPECS,
    OPS,
    _CUSTOM_DVE_ROW_BASE,
    _SUB_OPCODE_FOR_NAME,
    DveOp,
)
from concourse.dve_spec import C0, C1, C2, Spec, Src0, Src1, Zero, lower, maxx
from concourse.dve_uop import DveOpSpec

S = 2048
D = 256
P = 128
NJ = S // P  # 16
N_CORES = 8

f32 = mybir.dt.float32
f16 = mybir.dt.float16
Alu = mybir.AluOpType
Act = mybir.ActivationFunctionType


def _register_logits_op() -> DveOp:
    """Fused DVE op: out[p,k] = in0[p,k]*s0[p] - |in1[p,k] - s1[p]|*imm2.

    One instruction per key-chunk computes the full logits block
    (rank-1 qk product minus the scaled distance penalty).
    """
    name = "DWATT_LOGITS"
    existing = [op for op in OPS if op.name == name]
    if existing:
        return existing[0]
    d = Src1 - C1
    spec = Spec(
        body=Src0 * C0 - maxx(d, Zero - d) * C2,
        reference=lambda in0, in1, s0, s1, imm2: in0 * s0 - np.abs(in1 - s1) * imm2,
    )
    opcode = _CUSTOM_DVE_ROW_BASE + len(OPS)
    assert opcode < 0x20
    shas = {}
    for ver in ("v3", "v4"):
        try:
            shas[ver] = DveOpSpec(
                name=name, opcode=opcode, uops=lower(spec, ver=ver), rd1_en=True
            ).sha(ver)
        except Exception:
            pass
    op = DveOp(name, spec, subdim=False, uops_sha=shas)
    OPS.append(op)
    _SUB_OPCODE_FOR_NAME[name] = opcode
    CUSTOM_DVE_SPECS[name] = spec
    return op


DWATT_LOGITS = _register_logits_op()


def build_kernel(nc: bass.Bass, repeat: int = 1):
    # x is the per-batch input TRANSPOSED on host: [2, S], row 0 = sizes,
    # row 1 = heights (contiguous rows enable broadcast/column DMAs).
    x = nc.dram_tensor("x", [2, S], f16, kind="ExternalInput").ap()
    wq = nc.dram_tensor("wq", [1, D], f32, kind="ExternalInput").ap()
    wk = nc.dram_tensor("wk", [1, D], f32, kind="ExternalInput").ap()
    wv = nc.dram_tensor("wv", [1, D], f32, kind="ExternalInput").ap()
    out = nc.dram_tensor("out", [S, D], f32, kind="ExternalOutput").ap()

    with tile.TileContext(nc) as tc:
        from contextlib import ExitStack

        with ExitStack() as ctx:
            const_pool = ctx.enter_context(tc.tile_pool(name="const", bufs=1))
            work = ctx.enter_context(tc.tile_pool(name="work", bufs=4))
            epool = ctx.enter_context(tc.tile_pool(name="epool", bufs=3))
            qpool = ctx.enter_context(tc.tile_pool(name="qpool", bufs=12))
            mpsum = ctx.enter_context(
                tc.tile_pool(name="mpsum", bufs=1, space=bass.MemorySpace.PSUM)
            )
            cpsum = ctx.enter_context(
                tc.tile_pool(name="cpsum", bufs=1, space=bass.MemorySpace.PSUM)
            )
            for _rep in range(repeat):
                _kernel_body(nc, tc, const_pool, work, epool, qpool, mpsum, cpsum, x, wq, wk, wv, out)

    return nc


def _kernel_body(nc, tc, const_pool, work, epool, qpool, mpsum, cpsum, x, wq, wk, wv, out):
    if True:
        if True:
            # Tiny first load: scalar columns for key chunks {0,1,14,15}
            # (two consecutive-pair DMAs keep the APs 3D-balanced), so the
            # leading/trailing chunks never wait on the full column load.
            x_cols = x.rearrange("c (j p) -> p c j", p=P)
            colA = const_pool.tile([P, 2, 2], f16)
            nc.gpsimd.dma_start(colA[:, :, 0:1], x_cols[:, :, NJ - 1 : NJ])
            nc.gpsimd.dma_start(colA[:, :, 1:2], x_cols[:, :, NJ - 2 : NJ - 1])
            colfab = const_pool.tile([P, 4], f32)
            nc.vector.tensor_copy(colfab[:], colA[:].rearrange("p c j -> p (c j)"))
            # colfab: [sig_15, sig_14, h_15, h_14]

            col3 = const_pool.tile([P, 2, NJ], f16)
            nc.gpsimd.dma_start(col3[:], x.rearrange("c (j p) -> p c j", p=P))
            colh = col3[:].rearrange("p c j -> p (c j)")  # [:, :16]=sig, [:, 16:]=h
            # f32 copy: per-partition scalar operands must be float32
            colft = const_pool.tile([P, 2 * NJ], f32)
            nc.vector.tensor_copy(colft[:], colh)
            colf = colft[:]
            wv_rep = const_pool.tile([P, D], f32)
            nc.gpsimd.dma_start(wv_rep[:], wv.to_broadcast([P, D]))

            # Replicated rows (every partition holds the full row).
            Q = S // 4
            sig_rep = const_pool.tile([P, S], f16)
            h_rep = const_pool.tile([P, S], f16)
            qeng = [nc.sync, nc.scalar, nc.sync, nc.scalar]
            wq_t = const_pool.tile([P, D], f32)
            wk_t = const_pool.tile([P, D], f32)
            for q in range(4):
                lo, hi = Q * q, Q * (q + 1)
                qeng[q].dma_start(sig_rep[:, lo:hi], x[0:1, lo:hi].to_broadcast([P, Q]))
                qeng[q + 1 if q % 2 == 0 else q - 1].dma_start(
                    h_rep[:, lo:hi], x[1:2, lo:hi].to_broadcast([P, Q])
                )
                if q == 0:
                    # Wq/Wk pre-broadcast (c computed with pure DVE ops, no
                    # PE round trip) — queued after the first rep quarters.
                    nc.sync.dma_start(wq_t[:], wq.to_broadcast([P, D]))
                    nc.scalar.dma_start(wk_t[:], wk.to_broadcast([P, D]))

            # ---- c = (Wq . Wk) / 16 on every partition ------------------
            wqk = const_pool.tile([P, D], f32)
            nc.vector.tensor_mul(wqk[:], wq_t[:], wk_t[:])
            c_red = const_pool.tile([P, 1], f32)
            nc.vector.tensor_reduce(c_red[:], wqk[:], axis=mybir.AxisListType.X, op=Alu.add)
            c_col = const_pool.tile([P, 1], f32)
            nc.vector.tensor_scalar_mul(c_col[:], c_red[:], 1.0 / 16.0)
            # chAB: c*h for key chunks 15 and 14 (early); ch_col for rest
            chAB = const_pool.tile([P, 2], f32)
            nc.vector.tensor_scalar_mul(chAB[:], colfab[:, 2:4], c_col[:])
            # ch_col[p, j] = c * h[128*j + p]
            ch_col = const_pool.tile([P, NJ], f32)
            nc.vector.tensor_scalar_mul(ch_col[:], colf[:, NJ : 2 * NJ], c_col[:])

            # hones: cols 0..15 = h chunks (fp16), cols 16..31 = 1.0
            hones = const_pool.tile([P, 2 * NJ], f16)
            nc.vector.tensor_copy(hones[:, NJ - 2 : NJ - 1], colfab[:, 3:4])
            nc.vector.tensor_copy(hones[:, NJ - 1 : NJ], colfab[:, 2:3])
            nc.vector.tensor_copy(hones[:, 0 : NJ - 2], colh[:, NJ : 2 * NJ - 2])
            nc.vector.memset(hones[:, NJ : 2 * NJ], 1.0)

            # 2x2 identity (stationary for the num/den transpose matmuls)
            i2 = const_pool.tile([2, 2], f32)
            nc.gpsimd.memset(i2[:], 1.0)
            nc.gpsimd.affine_select(
                out=i2[:],
                in_=i2[:],
                compare_op=Alu.is_equal,
                fill=0.0,
                base=0,
                pattern=[[-1, 2]],
                channel_multiplier=1,
            )

            # ---- main loop over key chunks ------------------------------
            # psum rows: 0 = num[s] (sum_t h_t*E), 1 = den[s] (sum_t E).
            # Each 512-col slice is exactly one PSUM bank, so per-slice
            # start=(jt==0) resets only its own bank.
            psum_nd = mpsum.tile([2, S], f32)
            nd_sb = const_pool.tile([2, S], f32)
            psum_t = cpsum.tile([P, 2 * NJ], f32, tag="t")
            nc.vector.memset(psum_t[:], 0.0)

            # Quartered chunks run in 512-wide query quarters. jt=15 and
            # jt=0 go first (their scalars come from the tiny colA load and
            # each quarter only needs one replicated-row quarter, so the
            # scheduler can hoist them into the DMA window); jt=14 goes
            # last and carries the stop + the per-quarter num/den transpose
            # (4 small PE matmuls against I2 per quarter).
            def quarter_compute(sig_ap, ch_ap):
                tiles = []
                for q in range(4):
                    lo, hi = 512 * q, 512 * (q + 1)
                    lgq = qpool.tile([P, 512], f16, tag="lgq")
                    nc.vector._custom_dve(
                        DWATT_LOGITS,
                        out=lgq[:],
                        in0=h_rep[:, lo:hi],
                        in1=sig_rep[:, lo:hi],
                        s0=ch_ap,
                        s1=sig_ap,
                        imm2=0.5,
                    )
                    eeq = qpool.tile([P, 512], f16, tag="eeq")
                    nc.scalar.activation(eeq[:], lgq[:], Act.Exp)
                    tiles.append(eeq)
                return tiles

            def quarter_reduce(jtq, tiles, start, stop, tail):
                for q in range(4):
                    lo, hi = 512 * q, 512 * (q + 1)
                    nc.tensor.matmul(
                        psum_nd[:, lo:hi],
                        hones[:, jtq : jtq + NJ + 1 : NJ],
                        tiles[q][:],
                        start=start,
                        stop=stop,
                        skip_group_check=True,
                    )
                    if tail:
                        nc.vector.tensor_copy(nd_sb[:, lo:hi], psum_nd[:, lo:hi])
                        for j in range(4 * q, 4 * q + 4):
                            nc.tensor.matmul(
                                psum_t[:, 2 * j : 2 * j + 2],
                                nd_sb[:, P * j : P * (j + 1)],
                                i2[:],
                                start=False,
                                stop=(j == NJ - 1),
                                skip_group_check=True,
                            )

            for jt in range(0, NJ - 2):
                lg = work.tile([P, S], f16, tag="lg")
                nc.vector._custom_dve(
                    DWATT_LOGITS,
                    out=lg[:],
                    in0=h_rep[:],
                    in1=sig_rep[:],
                    s0=ch_col[:, jt : jt + 1],
                    s1=colf[:, jt : jt + 1],
                    imm2=0.5,
                )
                ee = epool.tile([P, S], f16, tag="ee")
                nc.scalar.activation(ee[:], lg[:], Act.Exp)
                for ks in range(S // 512):
                    nc.tensor.matmul(
                        psum_nd[:, 512 * ks : 512 * (ks + 1)],
                        hones[:, jt : jt + NJ + 1 : NJ],
                        ee[:, 512 * ks : 512 * (ks + 1)],
                        start=(jt == 0),
                        stop=False,
                        skip_group_check=True,
                    )

            jt14 = NJ - 2
            t14 = quarter_compute(colf[:, jt14 : jt14 + 1], ch_col[:, jt14 : jt14 + 1])
            quarter_reduce(NJ - 2, t14, False, False, False)
            t15 = quarter_compute(colfab[:, 0:1], chAB[:, 0:1])
            quarter_reduce(NJ - 1, t15, False, True, True)

            # ---- per-quarter: a = num/den, out chunks = a * Wv, DMA -----
            out_sb = const_pool.tile([P, NJ * D], f32)
            out_r = out.rearrange("(j p) d -> p j d", p=P)
            ob3 = out_sb[:].rearrange("p (j d) -> p j d", d=D)
            nd_t = const_pool.tile([P, 2 * NJ], f32)
            inv = const_pool.tile([P, NJ], f32)
            a_t = const_pool.tile([P, NJ], f32)
            for q in range(4):
                c8 = nd_t[:, 8 * q : 8 * q + 8]
                nc.vector.tensor_copy(c8, psum_t[:, 8 * q : 8 * q + 8])
                nc.vector.reciprocal(inv[:, 4 * q : 4 * q + 4], c8[:, 1:8:2])
                nc.vector.tensor_mul(
                    a_t[:, 4 * q : 4 * q + 4], c8[:, 0:8:2], inv[:, 4 * q : 4 * q + 4]
                )
                for j in range(4 * q, 4 * q + 4):
                    dst = out_sb[:, D * j : D * (j + 1)]
                    if j % 4 == 3 or j == 14:
                        nc.scalar.mul(dst, wv_rep[:], a_t[:, j : j + 1])
                    else:
                        nc.vector.tensor_scalar_mul(dst, wv_rep[:], a_t[:, j : j + 1])
                qeng[q].dma_start(
                    out_r[:, 4 * q : 4 * (q + 1)], ob3[:, 4 * q : 4 * (q + 1)]
                )


_NC = {}


def _get_nc(repeat: int = 1):
    if repeat not in _NC:
        nc = bacc.Bacc("TRN2", target_bir_lowering=False, debug=False, num_devices=N_CORES)
        build_kernel(nc, repeat)
        nc.compile()
        _NC[repeat] = nc
    return _NC[repeat]


def kernel(inputs: np.ndarray, Wq: np.ndarray, Wk: np.ndarray, Wv: np.ndarray) -> np.ndarray:
    assert inputs.shape == (N_CORES, S, 2), inputs.shape
    nc = _get_nc()
    wq = np.ascontiguousarray(Wq, dtype=np.float32)
    wk = np.ascontiguousarray(Wk, dtype=np.float32)
    wv = np.ascontiguousarray(Wv, dtype=np.float32)
    in_maps = [
        {
            "x": np.ascontiguousarray(np.asarray(inputs[b], dtype=np.float32).T.astype(np.float16)),
            "wq": wq,
            "wk": wk,
            "wv": wv,
        }
        for b in range(N_CORES)
    ]
    res = run_bass_kernel_spmd(nc, in_maps, core_ids=list(range(N_CORES)))
    return np.stack([r["out"] for r in res.results], axis=0)



# revision 12
# speedup vs baseline: 2.1701x; 2.1701x over previous
"""Distance-weighted self-attention on 8 Trainium2 NeuronCores.

The reference network is rank-1 in d_model:
  q = h*Wq, k = h*Wk, v = h*Wv  (h = heights column)
  logits[p,k] = c*h_p*h_k - 0.5*|sig_p - sig_k|,  c = (Wq.Wk)/sqrt(256)
  out[p,:]   = (sum_k softmax(logits)[p,k]*h_k) * Wv.

Key identity used here: with L(p) = {k : sig_k <= sig_p},
  exp(-0.5|sig_p - sig_k|) = e^{-0.5 sig_p} e^{+0.5 sig_k}   for k in L(p)
                           = e^{+0.5 sig_p} e^{-0.5 sig_k}   otherwise,
and since |c*h_p*h_k| <~ 0.05, exp(c h_p h_k) = 1 + c h_p h_k to ~1e-3.
Dividing num/den by e^{-0.5 sig_p} (a per-row constant that cancels):
  den'_p = A0 + a*A1 + E_p*(SU - (A4 + a*A5)),  a = c*h_p, E = e^{sig_p}
  num'_p = A1 + a*A2 + E_p*(SU' - (A5 + a*A6))
where Am(p) = sum_{k in L(p)} g_k h_k^m  (g = e^{+0.5 sig} for m<4,
e^{-0.5 sig} for m>=4) and SU/SU' come from the unmasked totals T.

So the only O(S^2) work is the 0/1 comparison mask (one 4x-mode DVE
tensor_scalar per key-chunk half) and tiny PE matmuls lhsT=mask[128,128] x
rhs=moments[128,8] accumulating A into PSUM [128,8] per query chunk.  Three
key chunks get +/-1 Sign masks on the scalar engine instead (algebraically
folded via a ones-matmul correction), balancing DVE/ACT.  The output outer
product a x Wv runs on the PE from a transposed a-row, DMAed per quarter so
the 2 MB writeback overlaps the second half's mask phase.
"""

import os
import sys

import numpy as np

for _p in ("/opt/trn_rl_repo", "/root/.axon_site/_ro/trn_rl_repo"):
    if os.path.isdir(_p) and _p not in sys.path:
        sys.path.append(_p)

import concourse.bacc as bacc
import concourse.bass as bass
import concourse.mybir as mybir
import concourse.tile as tile
from concourse.bass_utils import run_bass_kernel_spmd

S = 2048
D = 256
P = 128
NJ = S // P  # 16
N_CORES = 8
HALF = S // 2

f32 = mybir.dt.float32
f16 = mybir.dt.float16
Alu = mybir.AluOpType
Act = mybir.ActivationFunctionType

ACT_JS = (13, 14, 15)  # key chunks whose masks run on the scalar engine
DEBUG = True


def build_kernel(nc: bass.Bass):
    # x is the per-batch input TRANSPOSED on host: [2, S], row 0 = sizes,
    # row 1 = heights.
    x = nc.dram_tensor("x", [2, S], f16, kind="ExternalInput").ap()
    wq = nc.dram_tensor("wq", [1, D], f32, kind="ExternalInput").ap()
    wk = nc.dram_tensor("wk", [1, D], f32, kind="ExternalInput").ap()
    wv = nc.dram_tensor("wv", [1, D], f32, kind="ExternalInput").ap()
    out = nc.dram_tensor("out", [S, D], f32, kind="ExternalOutput").ap()
    dbg = nc.dram_tensor("dbg", [P, 288], f32, kind="ExternalOutput").ap() if DEBUG else None

    with tile.TileContext(nc) as tc:
        from contextlib import ExitStack

        with ExitStack() as ctx:
            const = ctx.enter_context(tc.tile_pool(name="const", bufs=1))
            mpool = ctx.enter_context(tc.tile_pool(name="mpool", bufs=4))
            apsum = ctx.enter_context(
                tc.tile_pool(name="apsum", bufs=1, space=bass.MemorySpace.PSUM)
            )
            opsum = ctx.enter_context(
                tc.tile_pool(name="opsum", bufs=2, space=bass.MemorySpace.PSUM)
            )
            tpsum = ctx.enter_context(
                tc.tile_pool(name="tpsum", bufs=2, space=bass.MemorySpace.PSUM)
            )
            _body(nc, tc, const, mpool, apsum, opsum, tpsum, x, wq, wk, wv, out, dbg)
    return nc


def _body(nc, tc, const, mpool, apsum, opsum, tpsum, x, wq, wk, wv, out, dbg):
    # ---- input DMAs -----------------------------------------------------
    col3 = const.tile([P, 2, NJ], f16)
    nc.sync.dma_start(col3[:], x.rearrange("c (j p) -> p c j", p=P))
    sig_rep = const.tile([P, S], f16)
    nc.scalar.dma_start(sig_rep[:, 0:HALF], x[0:1, 0:HALF].to_broadcast([P, HALF]))
    nc.scalar.dma_start(sig_rep[:, HALF:S], x[0:1, HALF:S].to_broadcast([P, HALF]))
    wq_t = const.tile([P, D], f32)
    wk_t = const.tile([P, D], f32)
    nc.sync.dma_start(wq_t[:], wq.to_broadcast([P, D]))
    nc.sync.dma_start(wk_t[:], wk.to_broadcast([P, D]))
    wv_rep = const.tile([P, D], f32)
    nc.sync.dma_start(wv_rep[:], wv.to_broadcast([P, D]))

    # ---- columns, c, activation-side 1D vectors -------------------------
    colf = const.tile([P, 2 * NJ], f32)
    nc.vector.tensor_copy(colf[:], col3[:].rearrange("p c j -> p (c j)"))
    sig_col = colf[:, 0:NJ]
    h_col = colf[:, NJ : 2 * NJ]
    negsig = const.tile([P, NJ], f32)
    nc.vector.tensor_scalar_mul(negsig[:], sig_col, -1.0)

    gp = const.tile([P, NJ], f32)
    gm = const.tile([P, NJ], f32)
    ecol = const.tile([P, NJ], f32)
    nc.scalar.activation(gp[:], sig_col, Act.Exp, scale=0.5)
    nc.scalar.activation(gm[:], sig_col, Act.Exp, scale=-0.5)
    nc.scalar.activation(ecol[:], sig_col, Act.Exp, scale=1.0)

    wqk = const.tile([P, D], f32)
    nc.vector.tensor_mul(wqk[:], wq_t[:], wk_t[:])
    c_red = const.tile([P, 1], f32)
    nc.vector.tensor_reduce(c_red[:], wqk[:], axis=mybir.AxisListType.X, op=Alu.add)
    c_col = const.tile([P, 1], f32)
    nc.vector.tensor_scalar_mul(c_col[:], c_red[:], 1.0 / 16.0)

    # ---- moments rhs tile: Mom[:, 8j+m] = g h^m at k = 128j+part --------
    h2 = const.tile([P, NJ], f32)
    nc.vector.tensor_mul(h2[:], h_col, h_col)
    mom = const.tile([P, 8 * NJ], f16)
    nc.gpsimd.memset(mom[:], 0.0)
    momv = mom[:].rearrange("p (j m) -> p j m", m=8)
    nc.vector.tensor_copy(momv[:, :, 0], gp[:])
    nc.vector.tensor_mul(momv[:, :, 1], gp[:], h_col)
    nc.vector.tensor_mul(momv[:, :, 2], gp[:], h2[:])
    nc.gpsimd.tensor_copy(momv[:, :, 4], gm[:])
    nc.gpsimd.tensor_mul(momv[:, :, 5], gm[:], h_col)
    nc.gpsimd.tensor_mul(momv[:, :, 6], gm[:], h2[:])
    # halved rhs for the +/-1 sign-mask chunks, and their ones-correction
    momh3 = const.tile([P, 8 * len(ACT_JS)], f16)
    nc.vector.tensor_scalar_mul(momh3[:], mom[:, 8 * ACT_JS[0] : 8 * (ACT_JS[-1] + 1)], 0.5)
    ones = const.tile([P, P], f16)
    nc.vector.memset(ones[:], 1.0)
    ident = const.tile([P, P], f16)
    nc.gpsimd.memset(ident[:], 1.0)
    nc.gpsimd.affine_select(
        out=ident[:],
        in_=ident[:],
        compare_op=Alu.is_equal,
        fill=0.0,
        base=0,
        pattern=[[-1, P]],
        channel_multiplier=1,
    )

    # ---- totals T_m (m = 0..6), replicated on every partition -----------
    psum_t = tpsum.tile([P, 7], f32, tag="pt")
    for j in range(NJ):
        nc.tensor.matmul(
            psum_t[:],
            ones[:],
            mom[:, 8 * j : 8 * j + 7],
            start=(j == 0),
            stop=(j == NJ - 1),
            skip_group_check=True,
        )
    t2 = const.tile([P, 7], f32)  # T_m / 2
    nc.vector.tensor_scalar_mul(t2[:], psum_t[:], 0.5)

    if dbg is not None:
        dbgsb = const.tile([P, 288], f32)
        nc.vector.tensor_copy(dbgsb[:, 0:7], t2[:])
        nc.vector.tensor_copy(dbgsb[:, 16:144], mom[:])

    # ---- per-query globals (column layout [P, NJ]) ----------------------
    a_col = const.tile([P, NJ], f32)
    nc.vector.tensor_scalar_mul(a_col[:], h_col, c_col[:])
    su2 = const.tile([P, NJ], f32)  # (T4 + a*T5)/2
    nc.vector.tensor_scalar(su2[:], a_col[:], t2[:, 5:6], t2[:, 4:5], op0=Alu.mult, op1=Alu.add)
    sup2 = const.tile([P, NJ], f32)  # (T5 + a*T6)/2
    nc.vector.tensor_scalar(sup2[:], a_col[:], t2[:, 6:7], t2[:, 5:6], op0=Alu.mult, op1=Alu.add)
    g1 = const.tile([P, NJ], f32)  # (T0 + a*T1)/2
    nc.vector.tensor_scalar(g1[:], a_col[:], t2[:, 1:2], t2[:, 0:1], op0=Alu.mult, op1=Alu.add)
    g2 = const.tile([P, NJ], f32)  # (T1 + a*T2)/2
    nc.vector.tensor_scalar(g2[:], a_col[:], t2[:, 2:3], t2[:, 1:2], op0=Alu.mult, op1=Alu.add)
    esu = const.tile([P, NJ], f32)
    nc.gpsimd.tensor_mul(esu[:], ecol[:], su2[:])
    nc.gpsimd.tensor_add(esu[:], esu[:], g1[:])
    esup = const.tile([P, NJ], f32)
    nc.gpsimd.tensor_mul(esup[:], ecol[:], sup2[:])
    nc.gpsimd.tensor_add(esup[:], esup[:], g2[:])

    # ---- sign masks for ACT_JS on the scalar engine (halves) ------------
    sgn = {}
    for j in ACT_JS:
        sgn[j] = const.tile([P, S], f16, name=f"sgn{j}", tag=f"sgn{j}")
    for h in range(2):
        lo, hi = HALF * h, HALF * (h + 1)
        for j in ACT_JS:
            nc.scalar.activation(
                sgn[j][:, lo:hi], sig_rep[:, lo:hi], Act.Sign, bias=negsig[:, j : j + 1]
            )

    # ---- main: masks + A matmuls, then per-half combine/output ----------
    # One PSUM tile (= one bank) per half: matmul start=True resets the
    # whole bank, so only the first matmul per bank may set it.
    out_r = out.rearrange("(i p) d -> p i d", p=P)
    qeng = [nc.sync, nc.scalar, nc.sync, nc.scalar]

    for h in range(2):
        lo = HALF * h
        psum_a = apsum.tile([P, 64], f32, tag=f"pa{h}", name=f"pa{h}")
        first = True
        for j in range(NJ):
            if j in ACT_JS:
                lhs = sgn[j]
                rhs = momh3[:, 8 * ACT_JS.index(j) : 8 * ACT_JS.index(j) + 8]
                off = lo
            else:
                m = mpool.tile([P, HALF], f16, tag="mask")
                nc.vector.tensor_scalar(
                    m[:],
                    sig_rep[:, lo : lo + HALF],
                    sig_col[:, j : j + 1],
                    0.5,
                    op0=Alu.is_ge,
                    op1=Alu.subtract,
                )
                lhs = m
                rhs = mom[:, 8 * j : 8 * j + 8]
                off = 0
            for il in range(8):
                nc.tensor.matmul(
                    psum_a[:, 8 * il : 8 * il + 8],
                    lhs[:, off + P * il : off + P * (il + 1)],
                    rhs,
                    start=first,
                    stop=(j == NJ - 1 and il == 7),
                    skip_group_check=True,
                )
                first = False

        # ---- combine for half h (column layout [P, 8]) ------------------
        acp = const.tile([P, 64], f32, name=f"acp{h}", tag=f"acp{h}")
        nc.vector.tensor_copy(acp[:], psum_a[:])
        A = acp[:].rearrange("p (i m) -> p m i", m=8)  # A[m][i-local]
        cs = slice(8 * h, 8 * (h + 1))
        eh, ah = ecol[:, cs], a_col[:, cs]

        def tt(eng, name, in0, in1, op):
            t = const.tile([P, 8], f32, name=name, tag=name + str(h))
            getattr(eng, "tensor_tensor")(t[:], in0, in1, op=op)
            return t

        w0 = tt(nc.gpsimd, "w0", eh, A[:, 4], Alu.mult)
        w1 = tt(nc.gpsimd, "w1", eh, A[:, 5], Alu.mult)
        w2 = tt(nc.gpsimd, "w2", eh, A[:, 6], Alu.mult)
        q0 = tt(nc.gpsimd, "q0", A[:, 0], w0[:], Alu.subtract)
        q1 = tt(nc.vector, "q1", A[:, 1], w1[:], Alu.subtract)
        q2 = tt(nc.vector, "q2", A[:, 2], w2[:], Alu.subtract)
        u1 = tt(nc.vector, "u1", ah, q1[:], Alu.mult)
        d1 = tt(nc.vector, "d1", q0[:], u1[:], Alu.add)
        den = tt(nc.vector, "dn", d1[:], esu[:, cs], Alu.add)
        z1 = tt(nc.gpsimd, "z1", ah, q2[:], Alu.mult)
        n1 = tt(nc.gpsimd, "n1", q1[:], z1[:], Alu.add)
        num = tt(nc.gpsimd, "nm", n1[:], esup[:, cs], Alu.add)
        inv = const.tile([P, 8], f32, name=f"inv{h}", tag=f"inv{h}")
        nc.vector.reciprocal_approx_fast(inv[:], den[:])
        aout = tt(nc.vector, "ao", num[:], inv[:], Alu.mult)
        if dbg is not None:
            nc.vector.tensor_copy(dbgsb[:, 144 + 64 * h : 144 + 64 * (h + 1)], acp[:])
            nc.vector.tensor_copy(dbgsb[:, 4 + 8 * h : 4 + 8 * (h + 1)], aout[:])

        # outer products out[128i+p, :] = a[p,i] * Wv via per-partition
        # scalar multiplies, split DVE/ACT/Pool; writeback per quarter.
        for q in range(2):
            ob = const.tile([P, 4 * D], f32, name=f"ob{h}{q}", tag=f"ob{2 * h + q}")
            for il4 in range(4):
                i = 8 * h + 4 * q + il4
                dst = ob[:, D * il4 : D * (il4 + 1)]
                sc = aout[:, 4 * q + il4 : 4 * q + il4 + 1]
                if il4 == 3:
                    nc.scalar.mul(dst, wv_rep[:], sc)
                elif il4 == 2:
                    nc.gpsimd.tensor_scalar_mul(dst, wv_rep[:], sc)
                else:
                    nc.vector.tensor_scalar_mul(dst, wv_rep[:], sc)
            qq = 2 * h + q
            qeng[qq].dma_start(
                out_r[:, 4 * qq : 4 * (qq + 1)],
                ob[:].rearrange("p (i d) -> p i d", d=D),
            )
    if dbg is not None:
        nc.sync.dma_start(dbg, dbgsb[:])


_NC = {}


def _get_nc():
    if "nc" not in _NC:
        nc = bacc.Bacc("TRN2", target_bir_lowering=False, debug=False, num_devices=N_CORES)
        build_kernel(nc)
        nc.compile()
        _NC["nc"] = nc
    return _NC["nc"]


def kernel(inputs: np.ndarray, Wq: np.ndarray, Wk: np.ndarray, Wv: np.ndarray) -> np.ndarray:
    assert inputs.shape == (N_CORES, S, 2), inputs.shape
    nc = _get_nc()
    wq = np.ascontiguousarray(Wq, dtype=np.float32)
    wk = np.ascontiguousarray(Wk, dtype=np.float32)
    wv = np.ascontiguousarray(Wv, dtype=np.float32)
    in_maps = [
        {
            "x": np.ascontiguousarray(np.asarray(inputs[b], dtype=np.float32).T.astype(np.float16)),
            "wq": wq,
            "wk": wk,
            "wv": wv,
        }
        for b in range(N_CORES)
    ]
    res = run_bass_kernel_spmd(nc, in_maps, core_ids=list(range(N_CORES)))
    return np.stack([r["out"] for r in res.results], axis=0)


# revision 13
# speedup vs baseline: 2.2118x; 1.0193x over previous
"""Distance-weighted self-attention on 8 Trainium2 NeuronCores.

The reference network is rank-1 in d_model:
  q = h*Wq, k = h*Wk, v = h*Wv  (h = heights column)
  logits[p,k] = c*h_p*h_k - 0.5*|sig_p - sig_k|,  c = (Wq.Wk)/sqrt(256)
  out[p,:]   = (sum_k softmax(logits)[p,k]*h_k) * Wv.

Key identity used here: with L(p) = {k : sig_k <= sig_p},
  exp(-0.5|sig_p - sig_k|) = e^{-0.5 sig_p} e^{+0.5 sig_k}   for k in L(p)
                           = e^{+0.5 sig_p} e^{-0.5 sig_k}   otherwise,
and since |c*h_p*h_k| <~ 0.05, exp(c h_p h_k) = 1 + c h_p h_k to ~1e-3.
Dividing num/den by e^{-0.5 sig_p} (a per-row constant that cancels):
  den'_p = A0 + a*A1 + E_p*(SU - (A4 + a*A5)),  a = c*h_p, E = e^{sig_p}
  num'_p = A1 + a*A2 + E_p*(SU' - (A5 + a*A6))
where Am(p) = sum_{k in L(p)} g_k h_k^m  (g = e^{+0.5 sig} for m<4,
e^{-0.5 sig} for m>=4) and SU/SU' come from the unmasked totals T.

So the only O(S^2) work is the 0/1 comparison mask (one 4x-mode DVE
tensor_scalar per key-chunk half) and tiny PE matmuls lhsT=mask[128,128] x
rhs=moments[128,8] accumulating A into PSUM [128,8] per query chunk.  Three
key chunks get +/-1 Sign masks on the scalar engine instead (algebraically
folded via a ones-matmul correction), balancing DVE/ACT.  The output outer
product a x Wv runs on the PE from a transposed a-row, DMAed per quarter so
the 2 MB writeback overlaps the second half's mask phase.
"""

import os
import sys

import numpy as np

for _p in ("/opt/trn_rl_repo", "/root/.axon_site/_ro/trn_rl_repo"):
    if os.path.isdir(_p) and _p not in sys.path:
        sys.path.append(_p)

import concourse.bacc as bacc
import concourse.bass as bass
import concourse.mybir as mybir
import concourse.tile as tile
from concourse.bass_utils import run_bass_kernel_spmd

S = 2048
D = 256
P = 128
NJ = S // P  # 16
N_CORES = 8
HALF = S // 2

f32 = mybir.dt.float32
f16 = mybir.dt.float16
Alu = mybir.AluOpType
Act = mybir.ActivationFunctionType

ACT_JS = (13, 14, 15)  # key chunks whose masks run on the scalar engine
DEBUG = False


def build_kernel(nc: bass.Bass):
    # x is the per-batch input TRANSPOSED on host: [2, S], row 0 = sizes,
    # row 1 = heights.
    x = nc.dram_tensor("x", [2, S], f16, kind="ExternalInput").ap()
    wq = nc.dram_tensor("wq", [1, D], f32, kind="ExternalInput").ap()
    wk = nc.dram_tensor("wk", [1, D], f32, kind="ExternalInput").ap()
    wv = nc.dram_tensor("wv", [1, D], f32, kind="ExternalInput").ap()
    out = nc.dram_tensor("out", [S, D], f32, kind="ExternalOutput").ap()
    dbg = nc.dram_tensor("dbg", [P, 288], f32, kind="ExternalOutput").ap() if DEBUG else None

    with tile.TileContext(nc) as tc:
        from contextlib import ExitStack

        with ExitStack() as ctx:
            const = ctx.enter_context(tc.tile_pool(name="const", bufs=1))
            mpool = ctx.enter_context(tc.tile_pool(name="mpool", bufs=4))
            apsum = ctx.enter_context(
                tc.tile_pool(name="apsum", bufs=1, space=bass.MemorySpace.PSUM)
            )
            opsum = ctx.enter_context(
                tc.tile_pool(name="opsum", bufs=2, space=bass.MemorySpace.PSUM)
            )
            tpsum = ctx.enter_context(
                tc.tile_pool(name="tpsum", bufs=2, space=bass.MemorySpace.PSUM)
            )
            _body(nc, tc, const, mpool, apsum, opsum, tpsum, x, wq, wk, wv, out, dbg)
    return nc


def _body(nc, tc, const, mpool, apsum, opsum, tpsum, x, wq, wk, wv, out, dbg):
    # ---- input DMAs -----------------------------------------------------
    col3 = const.tile([P, 2, NJ], f16)
    nc.sync.dma_start(col3[:], x.rearrange("c (j p) -> p c j", p=P))
    sig_rep = const.tile([P, S], f16)
    nc.scalar.dma_start(sig_rep[:, 0:HALF], x[0:1, 0:HALF].to_broadcast([P, HALF]))
    nc.scalar.dma_start(sig_rep[:, HALF:S], x[0:1, HALF:S].to_broadcast([P, HALF]))
    wq_t = const.tile([P, D], f32)
    wk_t = const.tile([P, D], f32)
    nc.sync.dma_start(wq_t[:], wq.to_broadcast([P, D]))
    nc.sync.dma_start(wk_t[:], wk.to_broadcast([P, D]))
    wv_rep = const.tile([P, D], f32)
    nc.sync.dma_start(wv_rep[:], wv.to_broadcast([P, D]))

    # ---- columns, c, activation-side 1D vectors -------------------------
    colf = const.tile([P, 2 * NJ], f32)
    nc.vector.tensor_copy(colf[:], col3[:].rearrange("p c j -> p (c j)"))
    sig_col = colf[:, 0:NJ]
    h_col = colf[:, NJ : 2 * NJ]
    negsig = const.tile([P, NJ], f32)
    nc.vector.tensor_scalar_mul(negsig[:], sig_col, -1.0)

    gp = const.tile([P, NJ], f32)
    gm = const.tile([P, NJ], f32)
    ecol = const.tile([P, NJ], f32)
    nc.scalar.activation(gp[:], sig_col, Act.Exp, scale=0.5)
    nc.scalar.activation(gm[:], sig_col, Act.Exp, scale=-0.5)
    nc.scalar.activation(ecol[:], sig_col, Act.Exp, scale=1.0)

    wqk = const.tile([P, D], f32)
    nc.vector.tensor_mul(wqk[:], wq_t[:], wk_t[:])
    c_red = const.tile([P, 1], f32)
    nc.vector.tensor_reduce(c_red[:], wqk[:], axis=mybir.AxisListType.X, op=Alu.add)
    c_col = const.tile([P, 1], f32)
    nc.vector.tensor_scalar_mul(c_col[:], c_red[:], 1.0 / 16.0)

    # ---- moments rhs tile: Mom[:, 8j+m] = g h^m at k = 128j+part --------
    h2 = const.tile([P, NJ], f32)
    nc.vector.tensor_mul(h2[:], h_col, h_col)
    mom = const.tile([P, 8 * NJ], f16)
    nc.gpsimd.memset(mom[:], 0.0)
    momv = mom[:].rearrange("p (j m) -> p j m", m=8)
    nc.vector.tensor_copy(momv[:, :, 0], gp[:])
    nc.vector.tensor_mul(momv[:, :, 1], gp[:], h_col)
    nc.vector.tensor_mul(momv[:, :, 2], gp[:], h2[:])
    nc.gpsimd.tensor_copy(momv[:, :, 4], gm[:])
    nc.gpsimd.tensor_mul(momv[:, :, 5], gm[:], h_col)
    nc.gpsimd.tensor_mul(momv[:, :, 6], gm[:], h2[:])
    # halved rhs for the +/-1 sign-mask chunks, and their ones-correction
    momh3 = const.tile([P, 8 * len(ACT_JS)], f16)
    nc.vector.tensor_scalar_mul(momh3[:], mom[:, 8 * ACT_JS[0] : 8 * (ACT_JS[-1] + 1)], 0.5)
    ones = const.tile([P, P], f16)
    nc.vector.memset(ones[:], 1.0)
    ident = const.tile([P, P], f16)
    nc.gpsimd.memset(ident[:], 1.0)
    nc.gpsimd.affine_select(
        out=ident[:],
        in_=ident[:],
        compare_op=Alu.is_equal,
        fill=0.0,
        base=0,
        pattern=[[-1, P]],
        channel_multiplier=1,
    )

    # ---- totals T_m (m = 0..6), replicated on every partition -----------
    psum_t = tpsum.tile([P, 7], f32, tag="pt")
    for j in range(NJ):
        nc.tensor.matmul(
            psum_t[:],
            ones[:],
            mom[:, 8 * j : 8 * j + 7],
            start=(j == 0),
            stop=(j == NJ - 1),
            skip_group_check=True,
        )
    t2 = const.tile([P, 7], f32)  # T_m / 2
    nc.vector.tensor_scalar_mul(t2[:], psum_t[:], 0.5)

    if dbg is not None:
        dbgsb = const.tile([P, 288], f32)
        nc.vector.tensor_copy(dbgsb[:, 0:7], t2[:])
        nc.vector.tensor_copy(dbgsb[:, 16:144], mom[:])

    # ---- per-query globals (column layout [P, NJ]) ----------------------
    a_col = const.tile([P, NJ], f32)
    nc.vector.tensor_scalar_mul(a_col[:], h_col, c_col[:])
    su2 = const.tile([P, NJ], f32)  # (T4 + a*T5)/2
    nc.vector.tensor_scalar(su2[:], a_col[:], t2[:, 5:6], t2[:, 4:5], op0=Alu.mult, op1=Alu.add)
    sup2 = const.tile([P, NJ], f32)  # (T5 + a*T6)/2
    nc.vector.tensor_scalar(sup2[:], a_col[:], t2[:, 6:7], t2[:, 5:6], op0=Alu.mult, op1=Alu.add)
    g1 = const.tile([P, NJ], f32)  # (T0 + a*T1)/2
    nc.vector.tensor_scalar(g1[:], a_col[:], t2[:, 1:2], t2[:, 0:1], op0=Alu.mult, op1=Alu.add)
    g2 = const.tile([P, NJ], f32)  # (T1 + a*T2)/2
    nc.vector.tensor_scalar(g2[:], a_col[:], t2[:, 2:3], t2[:, 1:2], op0=Alu.mult, op1=Alu.add)
    esu = const.tile([P, NJ], f32)
    nc.gpsimd.tensor_mul(esu[:], ecol[:], su2[:])
    nc.gpsimd.tensor_add(esu[:], esu[:], g1[:])
    esup = const.tile([P, NJ], f32)
    nc.gpsimd.tensor_mul(esup[:], ecol[:], sup2[:])
    nc.gpsimd.tensor_add(esup[:], esup[:], g2[:])

    # ---- sign masks for ACT_JS on the scalar engine (halves) ------------
    sgn = {}
    for j in ACT_JS:
        sgn[j] = const.tile([P, S], f16, name=f"sgn{j}", tag=f"sgn{j}")
    for h in range(2):
        lo, hi = HALF * h, HALF * (h + 1)
        for j in ACT_JS:
            nc.scalar.activation(
                sgn[j][:, lo:hi], sig_rep[:, lo:hi], Act.Sign, bias=negsig[:, j : j + 1]
            )

    # ---- main: masks + A matmuls, then per-half combine/output ----------
    # One PSUM tile (= one bank) per half: matmul start=True resets the
    # whole bank, so only the first matmul per bank may set it.
    out_r = out.rearrange("(i p) d -> p i d", p=P)
    qeng = [nc.sync, nc.scalar, nc.sync, nc.scalar]

    for h in range(2):
        lo = HALF * h
        psum_a = apsum.tile([P, 64], f32, tag=f"pa{h}", name=f"pa{h}")
        first = True
        for j in range(NJ):
            if j in ACT_JS:
                lhs = sgn[j]
                rhs = momh3[:, 8 * ACT_JS.index(j) : 8 * ACT_JS.index(j) + 8]
                off = lo
            else:
                m = mpool.tile([P, HALF], f16, tag="mask")
                nc.vector.tensor_scalar(
                    m[:],
                    sig_rep[:, lo : lo + HALF],
                    sig_col[:, j : j + 1],
                    0.5,
                    op0=Alu.is_ge,
                    op1=Alu.subtract,
                )
                lhs = m
                rhs = mom[:, 8 * j : 8 * j + 8]
                off = 0
            for il in range(8):
                nc.tensor.matmul(
                    psum_a[:, 8 * il : 8 * il + 8],
                    lhs[:, off + P * il : off + P * (il + 1)],
                    rhs,
                    start=first,
                    stop=(j == NJ - 1 and il == 7),
                    skip_group_check=True,
                )
                first = False

        # ---- combine for half h (column layout [P, 8]) ------------------
        acp = const.tile([P, 64], f32, name=f"acp{h}", tag=f"acp{h}")
        nc.vector.tensor_copy(acp[:], psum_a[:])
        A = acp[:].rearrange("p (i m) -> p m i", m=8)  # A[m][i-local]
        cs = slice(8 * h, 8 * (h + 1))
        eh, ah = ecol[:, cs], a_col[:, cs]

        def tt(eng, name, in0, in1, op):
            t = const.tile([P, 8], f32, name=name, tag=name + str(h))
            getattr(eng, "tensor_tensor")(t[:], in0, in1, op=op)
            return t

        w0 = tt(nc.gpsimd, "w0", eh, A[:, 4], Alu.mult)
        w1 = tt(nc.gpsimd, "w1", eh, A[:, 5], Alu.mult)
        w2 = tt(nc.gpsimd, "w2", eh, A[:, 6], Alu.mult)
        q0 = tt(nc.gpsimd, "q0", A[:, 0], w0[:], Alu.subtract)
        q1 = tt(nc.vector, "q1", A[:, 1], w1[:], Alu.subtract)
        q2 = tt(nc.vector, "q2", A[:, 2], w2[:], Alu.subtract)
        u1 = tt(nc.vector, "u1", ah, q1[:], Alu.mult)
        d1 = tt(nc.vector, "d1", q0[:], u1[:], Alu.add)
        den = tt(nc.vector, "dn", d1[:], esu[:, cs], Alu.add)
        z1 = tt(nc.gpsimd, "z1", ah, q2[:], Alu.mult)
        n1 = tt(nc.gpsimd, "n1", q1[:], z1[:], Alu.add)
        num = tt(nc.gpsimd, "nm", n1[:], esup[:, cs], Alu.add)
        inv = const.tile([P, 8], f32, name=f"inv{h}", tag=f"inv{h}")
        nc.vector.reciprocal_approx_fast(inv[:], den[:])
        aout = tt(nc.vector, "ao", num[:], inv[:], Alu.mult)
        if dbg is not None:
            nc.vector.tensor_copy(dbgsb[:, 144 + 64 * h : 144 + 64 * (h + 1)], acp[:])
            nc.vector.tensor_copy(dbgsb[:, 4 + 8 * h : 4 + 8 * (h + 1)], aout[:])

        # outer products out[128i+p, :] = a[p,i] * Wv via per-partition
        # scalar multiplies, split DVE/ACT/Pool; writeback per quarter.
        for q in range(2):
            ob = const.tile([P, 4 * D], f32, name=f"ob{h}{q}", tag=f"ob{2 * h + q}")
            for il4 in range(4):
                i = 8 * h + 4 * q + il4
                dst = ob[:, D * il4 : D * (il4 + 1)]
                sc = aout[:, 4 * q + il4 : 4 * q + il4 + 1]
                if il4 == 3:
                    nc.scalar.mul(dst, wv_rep[:], sc)
                elif il4 == 2:
                    nc.gpsimd.tensor_scalar_mul(dst, wv_rep[:], sc)
                else:
                    nc.vector.tensor_scalar_mul(dst, wv_rep[:], sc)
            qq = 2 * h + q
            qeng[qq].dma_start(
                out_r[:, 4 * qq : 4 * (qq + 1)],
                ob[:].rearrange("p (i d) -> p i d", d=D),
            )
    if dbg is not None:
        nc.sync.dma_start(dbg, dbgsb[:])


_NC = {}


def _get_nc():
    if "nc" not in _NC:
        nc = bacc.Bacc("TRN2", target_bir_lowering=False, debug=False, num_devices=N_CORES)
        build_kernel(nc)
        nc.compile()
        _NC["nc"] = nc
    return _NC["nc"]


def kernel(inputs: np.ndarray, Wq: np.ndarray, Wk: np.ndarray, Wv: np.ndarray) -> np.ndarray:
    assert inputs.shape == (N_CORES, S, 2), inputs.shape
    nc = _get_nc()
    wq = np.ascontiguousarray(Wq, dtype=np.float32)
    wk = np.ascontiguousarray(Wk, dtype=np.float32)
    wv = np.ascontiguousarray(Wv, dtype=np.float32)
    in_maps = [
        {
            "x": np.ascontiguousarray(np.asarray(inputs[b], dtype=np.float32).T.astype(np.float16)),
            "wq": wq,
            "wk": wk,
            "wv": wv,
        }
        for b in range(N_CORES)
    ]
    res = run_bass_kernel_spmd(nc, in_maps, core_ids=list(range(N_CORES)))
    return np.stack([r["out"] for r in res.results], axis=0)


# revision 15
# speedup vs baseline: 2.4460x; 1.1059x over previous
"""Distance-weighted self-attention on 8 Trainium2 NeuronCores.

The reference network is rank-1 in d_model:
  q = h*Wq, k = h*Wk, v = h*Wv  (h = heights column)
  logits[p,k] = c*h_p*h_k - 0.5*|sig_p - sig_k|,  c = (Wq.Wk)/sqrt(256)
  out[p,:]   = (sum_k softmax(logits)[p,k]*h_k) * Wv.

Key identity used here: with L(p) = {k : sig_k <= sig_p},
  exp(-0.5|sig_p - sig_k|) = e^{-0.5 sig_p} e^{+0.5 sig_k}   for k in L(p)
                           = e^{+0.5 sig_p} e^{-0.5 sig_k}   otherwise,
and since |c*h_p*h_k| <~ 0.05, exp(c h_p h_k) = 1 + c h_p h_k to ~1e-3.
Dividing num/den by e^{-0.5 sig_p} (a per-row constant that cancels):
  den'_p = A0 + a*A1 + E_p*(SU - (A4 + a*A5)),  a = c*h_p, E = e^{sig_p}
  num'_p = A1 + a*A2 + E_p*(SU' - (A5 + a*A6))
where Am(p) = sum_{k in L(p)} g_k h_k^m  (g = e^{+0.5 sig} for m<4,
e^{-0.5 sig} for m>=4) and SU/SU' come from the unmasked totals T.

So the only O(S^2) work is the 0/1 comparison mask (one 4x-mode DVE
tensor_scalar per key-chunk half) and tiny PE matmuls lhsT=mask[128,128] x
rhs=moments[128,8] accumulating A into PSUM [128,8] per query chunk.  Three
key chunks get +/-1 Sign masks on the scalar engine instead (algebraically
folded via a ones-matmul correction), balancing DVE/ACT.  The output outer
product a x Wv runs on the PE from a transposed a-row, DMAed per quarter so
the 2 MB writeback overlaps the second half's mask phase.
"""

import os
import sys

import numpy as np

for _p in ("/opt/trn_rl_repo", "/root/.axon_site/_ro/trn_rl_repo"):
    if os.path.isdir(_p) and _p not in sys.path:
        sys.path.append(_p)

import concourse.bacc as bacc
import concourse.bass as bass
import concourse.mybir as mybir
import concourse.tile as tile
from concourse.bass_utils import run_bass_kernel_spmd

S = 2048
D = 256
P = 128
NJ = S // P  # 16
N_CORES = 8
HALF = S // 2

f32 = mybir.dt.float32
f16 = mybir.dt.float16
Alu = mybir.AluOpType
Act = mybir.ActivationFunctionType

ACT_JS = (13, 14, 15)  # key chunks whose masks run on the scalar engine
DEBUG = False


def build_kernel(nc: bass.Bass):
    # x is the per-batch input TRANSPOSED on host: [2, S], row 0 = sizes,
    # row 1 = heights.
    x = nc.dram_tensor("x", [2, S], f16, kind="ExternalInput").ap()
    # host-pretransposed columns: xc[p, c*NJ + j] = x[c, j*128 + p]
    xc = nc.dram_tensor("xc", [P, 2 * NJ], f16, kind="ExternalInput").ap()
    wq = nc.dram_tensor("wq", [1, D], f32, kind="ExternalInput").ap()
    wk = nc.dram_tensor("wk", [1, D], f32, kind="ExternalInput").ap()
    wv = nc.dram_tensor("wv", [1, D], f32, kind="ExternalInput").ap()
    out = nc.dram_tensor("out", [S, D], f32, kind="ExternalOutput").ap()
    dbg = nc.dram_tensor("dbg", [P, 288], f32, kind="ExternalOutput").ap() if DEBUG else None

    with tile.TileContext(nc) as tc:
        from contextlib import ExitStack

        with ExitStack() as ctx:
            const = ctx.enter_context(tc.tile_pool(name="const", bufs=1))
            mpool = ctx.enter_context(tc.tile_pool(name="mpool", bufs=4))
            apsum = ctx.enter_context(
                tc.tile_pool(name="apsum", bufs=1, space=bass.MemorySpace.PSUM)
            )
            opsum = ctx.enter_context(
                tc.tile_pool(name="opsum", bufs=2, space=bass.MemorySpace.PSUM)
            )
            tpsum = ctx.enter_context(
                tc.tile_pool(name="tpsum", bufs=2, space=bass.MemorySpace.PSUM)
            )
            _body(nc, tc, const, mpool, apsum, opsum, tpsum, x, xc, wq, wk, wv, out, dbg)
    return nc


def _body(nc, tc, const, mpool, apsum, opsum, tpsum, x, xc, wq, wk, wv, out, dbg):
    # ---- act-table preload during the DMA window ------------------------
    dummy = const.tile([1, 1], f32)
    nc.vector.memset(dummy[:], 0.0)
    nc.scalar.activation(dummy[:], dummy[:], Act.Sign)

    # ---- input DMAs (sig_rep half 0 first: it gates the first masks) ----
    sig_rep = const.tile([P, S], f16)
    nc.scalar.dma_start(sig_rep[:, 0:HALF], x[0:1, 0:HALF].to_broadcast([P, HALF]))
    colq = const.tile([P, 2 * NJ], f16)
    nc.sync.dma_start(colq[:], xc)
    nc.scalar.dma_start(sig_rep[:, HALF:S], x[0:1, HALF:S].to_broadcast([P, HALF]))
    wq_r = const.tile([1, D], f32)
    wk_r = const.tile([1, D], f32)
    nc.sync.dma_start(wq_r[:], wq)
    nc.sync.dma_start(wk_r[:], wk)
    wv_rep = const.tile([P, D], f32)
    nc.sync.dma_start(wv_rep[:], wv.to_broadcast([P, D]))

    # ---- columns + activation-side 1D vectors ---------------------------
    colf = const.tile([P, 2 * NJ], f32)
    nc.vector.tensor_copy(colf[:], colq[:])
    sig_col = colf[:, 0:NJ]
    h_col = colf[:, NJ : 2 * NJ]
    negsig = const.tile([P, NJ], f32)
    nc.vector.tensor_scalar_mul(negsig[:], sig_col, -1.0)

    gp = const.tile([P, NJ], f32)
    gm = const.tile([P, NJ], f32)
    ecol = const.tile([P, NJ], f32)
    nc.scalar.activation(gp[:], sig_col, Act.Exp, scale=0.5)
    nc.scalar.activation(gm[:], sig_col, Act.Exp, scale=-0.5)
    nc.scalar.activation(ecol[:], sig_col, Act.Exp, scale=1.0)

    # c = (Wq.Wk)/16 on one partition, then PE-broadcast to all partitions
    wqk = const.tile([1, D], f32)
    nc.vector.tensor_mul(wqk[:], wq_r[:], wk_r[:])
    c_one = const.tile([1, 1], f32)
    nc.vector.tensor_reduce(c_one[:], wqk[:], axis=mybir.AxisListType.X, op=Alu.add)
    c_sc = const.tile([1, 1], f32)
    nc.vector.tensor_scalar_mul(c_sc[:], c_one[:], 1.0 / 16.0)

    # ---- moments rhs tile: Mom[:, 8j+m] = g h^m at k = 128j+part --------
    h2 = const.tile([P, NJ], f32)
    nc.vector.tensor_mul(h2[:], h_col, h_col)
    mom = const.tile([P, 8 * NJ], f16)
    nc.gpsimd.memset(mom[:], 0.0)
    momv = mom[:].rearrange("p (j m) -> p j m", m=8)
    nc.vector.tensor_copy(momv[:, :, 0], gp[:])
    nc.vector.tensor_mul(momv[:, :, 1], gp[:], h_col)
    nc.vector.tensor_mul(momv[:, :, 2], gp[:], h2[:])
    nc.gpsimd.tensor_copy(momv[:, :, 4], gm[:])
    nc.gpsimd.tensor_mul(momv[:, :, 5], gm[:], h_col)
    nc.gpsimd.tensor_mul(momv[:, :, 6], gm[:], h2[:])
    # halved rhs for the +/-1 sign-mask chunks, and their ones-correction
    momh3 = const.tile([P, 8 * len(ACT_JS)], f16)
    nc.vector.tensor_scalar_mul(momh3[:], mom[:, 8 * ACT_JS[0] : 8 * (ACT_JS[-1] + 1)], 0.5)
    ones = const.tile([P, P], f16)
    nc.vector.memset(ones[:], 1.0)
    onesf = const.tile([1, P], f32)
    nc.vector.memset(onesf[:], 1.0)
    psum_c = tpsum.tile([P, 1], f32, tag="pc")
    nc.tensor.matmul(psum_c[:], onesf[:], c_sc[:], start=True, stop=True, skip_group_check=True)
    c_col = const.tile([P, 1], f32)
    nc.vector.tensor_copy(c_col[:], psum_c[:])

    # ---- totals T_m (m = 0..6), replicated on every partition -----------
    psum_t = tpsum.tile([P, 7], f32, tag="pt")
    for j in range(NJ):
        nc.tensor.matmul(
            psum_t[:],
            ones[:],
            mom[:, 8 * j : 8 * j + 7],
            start=(j == 0),
            stop=(j == NJ - 1),
            skip_group_check=True,
        )
    t2 = const.tile([P, 7], f32)  # T_m / 2
    nc.vector.tensor_scalar_mul(t2[:], psum_t[:], 0.5)

    if dbg is not None:
        dbgsb = const.tile([P, 288], f32)
        nc.vector.tensor_copy(dbgsb[:, 0:7], t2[:])
        nc.vector.tensor_copy(dbgsb[:, 16:144], mom[:])

    # ---- per-query globals (column layout [P, NJ]) — all on Pool --------
    a_col = const.tile([P, NJ], f32)
    nc.gpsimd.tensor_scalar_mul(a_col[:], h_col, c_col[:])
    su2 = const.tile([P, NJ], f32)  # (T4 + a*T5)/2
    nc.gpsimd.tensor_scalar(su2[:], a_col[:], t2[:, 5:6], t2[:, 4:5], op0=Alu.mult, op1=Alu.add)
    sup2 = const.tile([P, NJ], f32)  # (T5 + a*T6)/2
    nc.gpsimd.tensor_scalar(sup2[:], a_col[:], t2[:, 6:7], t2[:, 5:6], op0=Alu.mult, op1=Alu.add)
    g1 = const.tile([P, NJ], f32)  # (T0 + a*T1)/2
    nc.gpsimd.tensor_scalar(g1[:], a_col[:], t2[:, 1:2], t2[:, 0:1], op0=Alu.mult, op1=Alu.add)
    g2 = const.tile([P, NJ], f32)  # (T1 + a*T2)/2
    nc.gpsimd.tensor_scalar(g2[:], a_col[:], t2[:, 2:3], t2[:, 1:2], op0=Alu.mult, op1=Alu.add)
    esu = const.tile([P, NJ], f32)
    nc.gpsimd.tensor_mul(esu[:], ecol[:], su2[:])
    nc.gpsimd.tensor_add(esu[:], esu[:], g1[:])
    esup = const.tile([P, NJ], f32)
    nc.gpsimd.tensor_mul(esup[:], ecol[:], sup2[:])
    nc.gpsimd.tensor_add(esup[:], esup[:], g2[:])

    # ---- sign masks for ACT_JS on the scalar engine (halves) ------------
    sgn = {}
    for j in ACT_JS:
        sgn[j] = const.tile([P, S], f16, name=f"sgn{j}", tag=f"sgn{j}")
    for h in range(2):
        lo, hi = HALF * h, HALF * (h + 1)
        for j in ACT_JS:
            nc.scalar.activation(
                sgn[j][:, lo:hi], sig_rep[:, lo:hi], Act.Sign, bias=negsig[:, j : j + 1]
            )

    # ---- main: masks + A matmuls, then per-half combine/output ----------
    # One PSUM tile (= one bank) per half: matmul start=True resets the
    # whole bank, so only the first matmul per bank may set it.
    out_r = out.rearrange("(i p) d -> p i d", p=P)
    qeng = [nc.sync, nc.scalar, nc.sync, nc.scalar]

    for h in range(2):
        lo = HALF * h
        psum_a = apsum.tile([P, 64], f32, tag=f"pa{h}", name=f"pa{h}")
        first = True
        for j in range(NJ):
            if j in ACT_JS:
                lhs = sgn[j]
                rhs = momh3[:, 8 * ACT_JS.index(j) : 8 * ACT_JS.index(j) + 8]
                off = lo
            else:
                m = mpool.tile([P, HALF], f16, tag="mask")
                nc.vector.tensor_scalar(
                    m[:],
                    sig_rep[:, lo : lo + HALF],
                    sig_col[:, j : j + 1],
                    0.5,
                    op0=Alu.is_ge,
                    op1=Alu.subtract,
                )
                lhs = m
                rhs = mom[:, 8 * j : 8 * j + 8]
                off = 0
            for il in range(8):
                nc.tensor.matmul(
                    psum_a[:, 8 * il : 8 * il + 8],
                    lhs[:, off + P * il : off + P * (il + 1)],
                    rhs,
                    start=first,
                    stop=(j == NJ - 1 and il == 7),
                    skip_group_check=True,
                )
                first = False

        # ---- combine for half h (column layout [P, 8]) ------------------
        acp = const.tile([P, 64], f32, name=f"acp{h}", tag=f"acp{h}")
        nc.scalar.copy(acp[:], psum_a[:])
        A = acp[:].rearrange("p (i m) -> p m i", m=8)  # A[m][i-local]
        cs = slice(8 * h, 8 * (h + 1))
        eh, ah = ecol[:, cs], a_col[:, cs]

        def tt(eng, name, in0, in1, op):
            t = const.tile([P, 8], f32, name=name, tag=name + str(h))
            getattr(eng, "tensor_tensor")(t[:], in0, in1, op=op)
            return t

        w0 = tt(nc.gpsimd, "w0", eh, A[:, 4], Alu.mult)
        w1 = tt(nc.gpsimd, "w1", eh, A[:, 5], Alu.mult)
        w2 = tt(nc.gpsimd, "w2", eh, A[:, 6], Alu.mult)
        q0 = tt(nc.gpsimd, "q0", A[:, 0], w0[:], Alu.subtract)
        q1 = tt(nc.gpsimd, "q1", A[:, 1], w1[:], Alu.subtract)
        q2 = tt(nc.gpsimd, "q2", A[:, 2], w2[:], Alu.subtract)
        u1 = tt(nc.vector, "u1", ah, q1[:], Alu.mult)
        d1 = tt(nc.vector, "d1", q0[:], u1[:], Alu.add)
        den = tt(nc.vector, "dn", d1[:], esu[:, cs], Alu.add)
        z1 = tt(nc.vector, "z1", ah, q2[:], Alu.mult)
        n1 = tt(nc.vector, "n1", q1[:], z1[:], Alu.add)
        num = tt(nc.vector, "nm", n1[:], esup[:, cs], Alu.add)
        inv = const.tile([P, 8], f32, name=f"inv{h}", tag=f"inv{h}")
        nc.vector.reciprocal_approx_fast(inv[:], den[:])
        aout = tt(nc.vector, "ao", num[:], inv[:], Alu.mult)
        if dbg is not None:
            nc.vector.tensor_copy(dbgsb[:, 144 + 64 * h : 144 + 64 * (h + 1)], acp[:])
            nc.vector.tensor_copy(dbgsb[:, 4 + 8 * h : 4 + 8 * (h + 1)], aout[:])

        # outer products out[128i+p, :] = a[p,i] * Wv via per-partition
        # scalar multiplies, split DVE/ACT/Pool; writeback per quarter.
        for q in range(2):
            ob = const.tile([P, 4 * D], f32, name=f"ob{h}{q}", tag=f"ob{2 * h + q}")
            for il4 in range(4):
                i = 8 * h + 4 * q + il4
                dst = ob[:, D * il4 : D * (il4 + 1)]
                sc = aout[:, 4 * q + il4 : 4 * q + il4 + 1]
                if il4 == 3:
                    nc.scalar.mul(dst, wv_rep[:], sc)
                elif il4 == 2:
                    nc.gpsimd.tensor_scalar_mul(dst, wv_rep[:], sc)
                else:
                    nc.vector.tensor_scalar_mul(dst, wv_rep[:], sc)
            qq = 2 * h + q
            qeng[qq].dma_start(
                out_r[:, 4 * qq : 4 * (qq + 1)],
                ob[:].rearrange("p (i d) -> p i d", d=D),
            )
    if dbg is not None:
        nc.sync.dma_start(dbg, dbgsb[:])


_NC = {}


def _get_nc():
    if "nc" not in _NC:
        nc = bacc.Bacc("TRN2", target_bir_lowering=False, debug=False, num_devices=N_CORES)
        build_kernel(nc)
        nc.compile()
        _NC["nc"] = nc
    return _NC["nc"]


def kernel(inputs: np.ndarray, Wq: np.ndarray, Wk: np.ndarray, Wv: np.ndarray) -> np.ndarray:
    assert inputs.shape == (N_CORES, S, 2), inputs.shape
    nc = _get_nc()
    wq = np.ascontiguousarray(Wq, dtype=np.float32)
    wk = np.ascontiguousarray(Wk, dtype=np.float32)
    wv = np.ascontiguousarray(Wv, dtype=np.float32)
    xs = [np.ascontiguousarray(np.asarray(inputs[b], dtype=np.float32).T.astype(np.float16)) for b in range(N_CORES)]
    # xc[p, c*NJ + j] = x[c, j*128 + p]
    xcs = [np.ascontiguousarray(xb.reshape(2, NJ, P).transpose(2, 0, 1).reshape(P, 2 * NJ)) for xb in xs]
    in_maps = [
        {
            "x": xs[b],
            "xc": xcs[b],
            "wq": wq,
            "wk": wk,
            "wv": wv,
        }
        for b in range(N_CORES)
    ]
    res = run_bass_kernel_spmd(nc, in_maps, core_ids=list(range(N_CORES)))
    return np.stack([r["out"] for r in res.results], axis=0)


# revision 18
# speedup vs baseline: 2.7171x; 1.1108x over previous
"""Distance-weighted self-attention on 8 Trainium2 NeuronCores.

The reference network is rank-1 in d_model:
  q = h*Wq, k = h*Wk, v = h*Wv  (h = heights column)
  logits[p,k] = c*h_p*h_k - 0.5*|sig_p - sig_k|,  c = (Wq.Wk)/sqrt(256)
  out[p,:]   = (sum_k softmax(logits)[p,k]*h_k) * Wv.

Key identity used here: with L(p) = {k : sig_k <= sig_p},
  exp(-0.5|sig_p - sig_k|) = e^{-0.5 sig_p} e^{+0.5 sig_k}   for k in L(p)
                           = e^{+0.5 sig_p} e^{-0.5 sig_k}   otherwise,
and since |c*h_p*h_k| <~ 0.05, exp(c h_p h_k) = 1 + c h_p h_k to ~1e-3.
With the +-1/2 step convention s(p,k) = [sig_k <= sig_p] - 1/2 (ties -> 0,
exact because both branch formulas agree at sig_k == sig_p), the masked
sums A_m(p) = sum_k s(p,k) g_k h_k^m obey A_m = P_m + T_m/2 with P from
the sign-mask matmuls and T the plain totals.  Dividing num/den by
e^{-0.5 sig_p} (cancels in the ratio):
  den'_p = Q0 + a*Q1 + E*su2 + g1,  Q_m = P_m - E*P_{m+4}, E = e^{sig_p}
  num'_p = Q1 + a*Q2 + E*sup2 + g2
so the only O(S^2) device work is one 4x-mode DVE compare per key-chunk
half (3 chunks use +-1 Sign masks on the scalar engine) and tiny PE
matmuls lhsT=mask[128,128] x rhs=moments[128,8] accumulated into a
[128,8]-per-query-chunk PSUM bank (matmul start=True resets a whole PSUM
bank, so each half owns one bank and only its first matmul sets start).

The output outer product a x Wv runs as per-partition-scalar multiplies
into fp16, written back per quarter (host upcasts to f32), so the 1 MB
writeback overlaps the second half's mask phase.  Engines execute their
streams strictly in order, so emission order is laid out explicitly: the
scalar engine interleaves its PSUM->SBUF copy between the two sign-mask
halves, and the vector engine interleaves the half-0 combine into the
half-1 mask stream.
"""

import os
import sys

import numpy as np

for _p in ("/opt/trn_rl_repo", "/root/.axon_site/_ro/trn_rl_repo"):
    if os.path.isdir(_p) and _p not in sys.path:
        sys.path.append(_p)

import concourse.bacc as bacc
import concourse.bass as bass
import concourse.mybir as mybir
import concourse.tile as tile
from concourse.bass_utils import run_bass_kernel_spmd

S = 2048
D = 256
P = 128
NJ = S // P  # 16
N_CORES = 8
HALF = S // 2

f32 = mybir.dt.float32
f16 = mybir.dt.float16
Alu = mybir.AluOpType
Act = mybir.ActivationFunctionType

ACT_JS = (13, 14, 15)  # key chunks whose masks run on the scalar engine
DVE_JS = tuple(j for j in range(NJ) if j not in ACT_JS)


def build_kernel(nc: bass.Bass):
    x = nc.dram_tensor("x", [2, S], f16, kind="ExternalInput").ap()
    # host-pretransposed columns: xc[p, c*NJ + j] = x[c, j*128 + p]
    xc = nc.dram_tensor("xc", [P, 2 * NJ], f16, kind="ExternalInput").ap()
    wq = nc.dram_tensor("wq", [1, D], f32, kind="ExternalInput").ap()
    wk = nc.dram_tensor("wk", [1, D], f32, kind="ExternalInput").ap()
    wv16 = nc.dram_tensor("wv16", [1, D], f16, kind="ExternalInput").ap()
    out = nc.dram_tensor("out", [S, D], f16, kind="ExternalOutput").ap()

    with tile.TileContext(nc) as tc:
        from contextlib import ExitStack

        with ExitStack() as ctx:
            const = ctx.enter_context(tc.tile_pool(name="const", bufs=1))
            mpool = ctx.enter_context(tc.tile_pool(name="mpool", bufs=8))
            apsum = ctx.enter_context(
                tc.tile_pool(name="apsum", bufs=1, space=bass.MemorySpace.PSUM)
            )
            tpsum = ctx.enter_context(
                tc.tile_pool(name="tpsum", bufs=1, space=bass.MemorySpace.PSUM)
            )
            _body(nc, tc, const, mpool, apsum, tpsum, x, xc, wq, wk, wv16, out)
    return nc


def _body(nc, tc, const, mpool, apsum, tpsum, x, xc, wq, wk, wv16, out):
    # ---- DMAs: sig_rep half 0 gates the first masks — it goes first -----
    sig_rep = const.tile([P, S], f16)
    nc.sync.dma_start(sig_rep[:, 0:HALF], x[0:1, 0:HALF].to_broadcast([P, HALF]))
    colq = const.tile([P, 2 * NJ], f16)
    nc.scalar.dma_start(colq[:], xc)
    nc.sync.dma_start(sig_rep[:, HALF:S], x[0:1, HALF:S].to_broadcast([P, HALF]))
    wq_t = const.tile([P, D], f32)
    wk_t = const.tile([P, D], f32)
    nc.sync.dma_start(wq_t[:], wq.to_broadcast([P, D]))
    nc.sync.dma_start(wk_t[:], wk.to_broadcast([P, D]))
    wv_rep = const.tile([P, D], f16)
    nc.sync.dma_start(wv_rep[:], wv16.to_broadcast([P, D]))

    # ---- ACT: act-table preload during the DMA window -------------------
    dummy = const.tile([1, 1], f32)
    nc.vector.memset(dummy[:], 0.0)
    nc.scalar.activation(dummy[:], dummy[:], Act.Sign)

    # ---- DVE prologue: columns then straight into masks -----------------
    ones = const.tile([P, P], f16)
    nc.vector.memset(ones[:], 1.0)
    colf = const.tile([P, 2 * NJ], f32)
    nc.vector.tensor_copy(colf[:], colq[:])
    sig_col = colf[:, 0:NJ]
    h_col = colf[:, NJ : 2 * NJ]
    negsig = const.tile([P, NJ], f32)
    nc.vector.tensor_scalar_mul(negsig[:], sig_col, -1.0)

    gp = const.tile([P, NJ], f32)
    gm = const.tile([P, NJ], f32)
    ecol = const.tile([P, NJ], f32)
    nc.scalar.activation(gp[:], sig_col, Act.Exp, scale=0.5)
    nc.scalar.activation(gm[:], sig_col, Act.Exp, scale=-0.5)
    nc.scalar.activation(ecol[:], sig_col, Act.Exp, scale=1.0)

    # ---- Pool prologue: moments, c-chain --------------------------------
    mom = const.tile([P, 8 * NJ], f16)
    nc.gpsimd.memset(mom[:], 0.0)
    h2 = const.tile([P, NJ], f32)
    nc.gpsimd.tensor_mul(h2[:], h_col, h_col)
    momv = mom[:].rearrange("p (j m) -> p j m", m=8)
    nc.gpsimd.tensor_copy(momv[:, :, 0], gp[:])
    nc.gpsimd.tensor_mul(momv[:, :, 1], gp[:], h_col)
    nc.gpsimd.tensor_mul(momv[:, :, 2], gp[:], h2[:])
    nc.gpsimd.tensor_copy(momv[:, :, 4], gm[:])
    nc.gpsimd.tensor_mul(momv[:, :, 5], gm[:], h_col)
    nc.gpsimd.tensor_mul(momv[:, :, 6], gm[:], h2[:])
    momh3 = const.tile([P, 8 * len(ACT_JS)], f16)
    nc.gpsimd.tensor_scalar_mul(momh3[:], mom[:, 8 * ACT_JS[0] : 8 * (ACT_JS[-1] + 1)], 0.5)

    wqk = const.tile([P, D], f32)
    nc.gpsimd.tensor_mul(wqk[:], wq_t[:], wk_t[:])
    h16 = const.tile([P, NJ], f32)  # h/16 so a = (h/16) * (Wq.Wk)
    nc.gpsimd.tensor_scalar_mul(h16[:], h_col, 1.0 / 16.0)

    # ---- PE: totals -----------------------------------------------------
    psum_t = tpsum.tile([P, 7], f32, tag="pt")
    for j in range(NJ):
        nc.tensor.matmul(
            psum_t[:],
            ones[:],
            mom[:, 8 * j : 8 * j + 7],
            start=(j == 0),
            stop=(j == NJ - 1),
            skip_group_check=True,
        )

    # ---- Pool: per-query globals (t2/c_red filled by DVE post-h0-masks) -
    t2 = const.tile([P, 7], f32)  # T_m / 2
    c_red = const.tile([P, 1], f32)
    a_col = const.tile([P, NJ], f32)
    su2 = const.tile([P, NJ], f32)  # (T4 + a*T5)/2
    sup2 = const.tile([P, NJ], f32)  # (T5 + a*T6)/2
    g1 = const.tile([P, NJ], f32)  # (T0 + a*T1)/2
    g2 = const.tile([P, NJ], f32)  # (T1 + a*T2)/2
    esu = const.tile([P, NJ], f32)
    esup = const.tile([P, NJ], f32)

    def pool_globals():
        nc.gpsimd.tensor_scalar_mul(a_col[:], h16[:], c_red[:])
        nc.gpsimd.tensor_scalar(su2[:], a_col[:], t2[:, 5:6], t2[:, 4:5], op0=Alu.mult, op1=Alu.add)
        nc.gpsimd.tensor_scalar(sup2[:], a_col[:], t2[:, 6:7], t2[:, 5:6], op0=Alu.mult, op1=Alu.add)
        nc.gpsimd.tensor_scalar(g1[:], a_col[:], t2[:, 1:2], t2[:, 0:1], op0=Alu.mult, op1=Alu.add)
        nc.gpsimd.tensor_scalar(g2[:], a_col[:], t2[:, 2:3], t2[:, 1:2], op0=Alu.mult, op1=Alu.add)
        nc.gpsimd.tensor_mul(esu[:], ecol[:], su2[:])
        nc.gpsimd.tensor_add(esu[:], esu[:], g1[:])
        nc.gpsimd.tensor_mul(esup[:], ecol[:], sup2[:])
        nc.gpsimd.tensor_add(esup[:], esup[:], g2[:])

    # ---- masks and matmuls ----------------------------------------------
    sgn = {}
    for j in ACT_JS:
        sgn[j] = const.tile([P, S], f16, name=f"sgn{j}", tag=f"sgn{j}")
    psum_a = {}
    mstate = {}
    for h in range(2):
        psum_a[h] = apsum.tile([P, 64], f32, tag=f"pa{h}", name=f"pa{h}")
        mstate[h] = {"first": True}

    def act_sgn_half(h):
        lo, hi = HALF * h, HALF * (h + 1)
        for j in ACT_JS:
            nc.scalar.activation(
                sgn[j][:, lo:hi], sig_rep[:, lo:hi], Act.Sign, bias=negsig[:, j : j + 1]
            )

    def dve_mask(h, j):
        lo = HALF * h
        m = mpool.tile([P, HALF], f16, tag="mask")
        nc.vector.tensor_scalar(
            m[:],
            sig_rep[:, lo : lo + HALF],
            sig_col[:, j : j + 1],
            0.5,
            op0=Alu.is_ge,
            op1=Alu.subtract,
        )
        st = mstate[h]
        for il in range(8):
            nc.tensor.matmul(
                psum_a[h][:, 8 * il : 8 * il + 8],
                m[:, P * il : P * (il + 1)],
                mom[:, 8 * j : 8 * j + 8],
                start=st["first"],
                stop=False,
                skip_group_check=True,
            )
            st["first"] = False

    def act_matmuls(h):
        lo = HALF * h
        for jx, j in enumerate(ACT_JS):
            for il in range(8):
                last = jx == len(ACT_JS) - 1 and il == 7
                nc.tensor.matmul(
                    psum_a[h][:, 8 * il : 8 * il + 8],
                    sgn[j][:, lo + P * il : lo + P * (il + 1)],
                    momh3[:, 8 * jx : 8 * jx + 8],
                    start=False,
                    stop=last,
                    skip_group_check=True,
                )

    out_r = out.rearrange("(i p) d -> p i d", p=P)
    comb = {}

    def acopy(h):
        acp = const.tile([P, 64], f32, name=f"acp{h}", tag=f"acp{h}")
        nc.scalar.copy(acp[:], psum_a[h][:])
        comb[h] = {"acp": acp}

    def pool_combine(h):
        acp = comb[h]["acp"]
        A = acp[:].rearrange("p (i m) -> p m i", m=8)
        cs = slice(8 * h, 8 * (h + 1))
        eh = ecol[:, cs]

        def ptt(name, in0, in1, op):
            t = const.tile([P, 8], f32, name=name + str(h), tag=name + str(h))
            nc.gpsimd.tensor_tensor(t[:], in0, in1, op=op)
            return t

        w0 = ptt("w0", eh, A[:, 4], Alu.mult)
        w1 = ptt("w1", eh, A[:, 5], Alu.mult)
        w2 = ptt("w2", eh, A[:, 6], Alu.mult)
        comb[h]["q0"] = ptt("q0", A[:, 0], w0[:], Alu.subtract)
        comb[h]["q1"] = ptt("q1", A[:, 1], w1[:], Alu.subtract)
        comb[h]["q2"] = ptt("q2", A[:, 2], w2[:], Alu.subtract)

    def dve_combine(h):
        cs = slice(8 * h, 8 * (h + 1))
        ah = a_col[:, cs]
        q0, q1, q2 = comb[h]["q0"], comb[h]["q1"], comb[h]["q2"]

        def vtt(name, in0, in1, op):
            t = const.tile([P, 8], f32, name=name + str(h), tag=name + str(h))
            nc.vector.tensor_tensor(t[:], in0, in1, op=op)
            return t

        u1 = vtt("u1", ah, q1[:], Alu.mult)
        d1 = vtt("d1", q0[:], u1[:], Alu.add)
        den = vtt("dn", d1[:], esu[:, cs], Alu.add)
        z1 = vtt("z1", ah, q2[:], Alu.mult)
        n1 = vtt("n1", q1[:], z1[:], Alu.add)
        num = vtt("nm", n1[:], esup[:, cs], Alu.add)
        inv = const.tile([P, 8], f32, name=f"inv{h}", tag=f"inv{h}")
        nc.vector.reciprocal_approx_fast(inv[:], den[:])
        comb[h]["aout"] = vtt("ao", num[:], inv[:], Alu.mult)

    def outers_quarter(qq, engines):
        h = qq // 2
        q = qq % 2
        aout = comb[h]["aout"]
        ob = const.tile([P, 4 * D], f16, name=f"ob{qq}", tag=f"ob{qq}")
        for il4 in range(4):
            sc = aout[:, 4 * q + il4 : 4 * q + il4 + 1]
            dst = ob[:, D * il4 : D * (il4 + 1)]
            engines[il4].tensor_scalar_mul(dst, wv_rep[:], sc)
        return ob

    def out_dma(qq, ob):
        nc.sync.dma_start(
            out_r[:, 4 * qq : 4 * (qq + 1)],
            ob[:].rearrange("p (i d) -> p i d", d=D),
        )

    OE = [nc.vector, nc.vector, nc.gpsimd, nc.gpsimd]

    # ---- emission: h0 masks ---------------------------------------------
    act_sgn_half(0)  # ACT: sgn h0 pieces (after exps)
    for j in DVE_JS:
        dve_mask(0, j)
    act_matmuls(0)  # PE: ACT-mask matmuls h0 (incl stop)
    # DVE: psum-sourced scalars (psum_t stopped long ago; no stall here)
    nc.vector.tensor_scalar_mul(t2[:], psum_t[:], 0.5)
    nc.vector.tensor_reduce(c_red[:], wqk[:], axis=mybir.AxisListType.X, op=Alu.add)
    acopy(0)  # ACT: psum->sbuf for h0 (before sgn h1 in ACT order)
    act_sgn_half(1)  # ACT: sgn h1 pieces
    pool_globals()  # Pool: a, su2/sup2/g1/g2, esu/esup (needs t2, c_red)
    pool_combine(0)  # Pool: w/q for h0

    # ---- h1 masks part 1, then h0 combine + first outputs ---------------
    for j in DVE_JS[:6]:
        dve_mask(1, j)
    dve_combine(0)
    out_dma(0, outers_quarter(0, OE))
    out_dma(1, outers_quarter(1, OE))

    # ---- h1 masks part 2, close h1, outputs -----------------------------
    for j in DVE_JS[6:]:
        dve_mask(1, j)
    act_matmuls(1)
    acopy(1)  # ACT
    pool_combine(1)  # Pool
    dve_combine(1)
    out_dma(2, outers_quarter(2, OE))
    out_dma(3, outers_quarter(3, OE))


_NC = {}


def _get_nc():
    if "nc" not in _NC:
        nc = bacc.Bacc("TRN2", target_bir_lowering=False, debug=False, num_devices=N_CORES)
        build_kernel(nc)
        nc.compile()
        _NC["nc"] = nc
    return _NC["nc"]


def kernel(inputs: np.ndarray, Wq: np.ndarray, Wk: np.ndarray, Wv: np.ndarray) -> np.ndarray:
    assert inputs.shape == (N_CORES, S, 2), inputs.shape
    nc = _get_nc()
    wq = np.ascontiguousarray(Wq, dtype=np.float32)
    wk = np.ascontiguousarray(Wk, dtype=np.float32)
    wv16 = np.ascontiguousarray(np.asarray(Wv, dtype=np.float32).astype(np.float16))
    xs = [
        np.ascontiguousarray(np.asarray(inputs[b], dtype=np.float32).T.astype(np.float16))
        for b in range(N_CORES)
    ]
    # xc[p, c*NJ + j] = x[c, j*128 + p]
    xcs = [
        np.ascontiguousarray(xb.reshape(2, NJ, P).transpose(2, 0, 1).reshape(P, 2 * NJ))
        for xb in xs
    ]
    in_maps = [
        {"x": xs[b], "xc": xcs[b], "wq": wq, "wk": wk, "wv16": wv16}
        for b in range(N_CORES)
    ]
    res = run_bass_kernel_spmd(nc, in_maps, core_ids=list(range(N_CORES)))
    return np.stack([r["out"].astype(np.float32) for r in res.results], axis=0)


# revision 19
# speedup vs baseline: 2.8138x; 1.0356x over previous
"""Distance-weighted self-attention on 8 Trainium2 NeuronCores.

The reference network is rank-1 in d_model:
  q = h*Wq, k = h*Wk, v = h*Wv  (h = heights column)
  logits[p,k] = c*h_p*h_k - 0.5*|sig_p - sig_k|,  c = (Wq.Wk)/sqrt(256)
  out[p,:]   = (sum_k softmax(logits)[p,k]*h_k) * Wv.

Key identity used here: with L(p) = {k : sig_k <= sig_p},
  exp(-0.5|sig_p - sig_k|) = e^{-0.5 sig_p} e^{+0.5 sig_k}   for k in L(p)
                           = e^{+0.5 sig_p} e^{-0.5 sig_k}   otherwise,
and since |c*h_p*h_k| <~ 0.05, exp(c h_p h_k) = 1 + c h_p h_k to ~1e-3.
With the +-1/2 step convention s(p,k) = [sig_k <= sig_p] - 1/2 (ties -> 0,
exact because both branch formulas agree at sig_k == sig_p), the masked
sums A_m(p) = sum_k s(p,k) g_k h_k^m obey A_m = P_m + T_m/2 with P from
the sign-mask matmuls and T the plain totals.  Dividing num/den by
e^{-0.5 sig_p} (cancels in the ratio):
  den'_p = Q0 + a*Q1 + E*su2 + g1,  Q_m = P_m - E*P_{m+4}, E = e^{sig_p}
  num'_p = Q1 + a*Q2 + E*sup2 + g2
so the only O(S^2) device work is one 4x-mode DVE compare per key-chunk
half (3 chunks use +-1 Sign masks on the scalar engine) and tiny PE
matmuls lhsT=mask[128,128] x rhs=moments[128,8] accumulated into a
[128,8]-per-query-chunk PSUM bank (matmul start=True resets a whole PSUM
bank, so each half owns one bank and only its first matmul sets start).

The output outer product a x Wv runs as per-partition-scalar multiplies
into fp16, written back per quarter (host upcasts to f32), so the 1 MB
writeback overlaps the second half's mask phase.  Engines execute their
streams strictly in order, so emission order is laid out explicitly: the
scalar engine interleaves its PSUM->SBUF copy between the two sign-mask
halves, and the vector engine interleaves the half-0 combine into the
half-1 mask stream.
"""

import os
import sys

import numpy as np

for _p in ("/opt/trn_rl_repo", "/root/.axon_site/_ro/trn_rl_repo"):
    if os.path.isdir(_p) and _p not in sys.path:
        sys.path.append(_p)

import concourse.bacc as bacc
import concourse.bass as bass
import concourse.mybir as mybir
import concourse.tile as tile
from concourse.bass_utils import run_bass_kernel_spmd

S = 2048
D = 256
P = 128
NJ = S // P  # 16
N_CORES = 8
HALF = S // 2

f32 = mybir.dt.float32
f16 = mybir.dt.float16
Alu = mybir.AluOpType
Act = mybir.ActivationFunctionType

ACT_JS = (13, 14, 15)  # key chunks whose masks run on the scalar engine
POOL_H1_JS = (12,)  # key chunks whose half-1 masks run on gpsimd
DVE_JS = tuple(j for j in range(NJ) if j not in ACT_JS)
DVE_H1_JS = tuple(j for j in DVE_JS if j not in POOL_H1_JS)


def build_kernel(nc: bass.Bass):
    x = nc.dram_tensor("x", [2, S], f16, kind="ExternalInput").ap()
    # host-pretransposed columns: xc[p, c*NJ + j] = x[c, j*128 + p]
    xc = nc.dram_tensor("xc", [P, 2 * NJ], f16, kind="ExternalInput").ap()
    wq = nc.dram_tensor("wq", [1, D], f32, kind="ExternalInput").ap()
    wk = nc.dram_tensor("wk", [1, D], f32, kind="ExternalInput").ap()
    wv16 = nc.dram_tensor("wv16", [1, D], f16, kind="ExternalInput").ap()
    out = nc.dram_tensor("out", [S, D], f16, kind="ExternalOutput").ap()

    with tile.TileContext(nc) as tc:
        from contextlib import ExitStack

        with ExitStack() as ctx:
            const = ctx.enter_context(tc.tile_pool(name="const", bufs=1))
            mpool = ctx.enter_context(tc.tile_pool(name="mpool", bufs=8))
            apsum = ctx.enter_context(
                tc.tile_pool(name="apsum", bufs=1, space=bass.MemorySpace.PSUM)
            )
            tpsum = ctx.enter_context(
                tc.tile_pool(name="tpsum", bufs=1, space=bass.MemorySpace.PSUM)
            )
            _body(nc, tc, const, mpool, apsum, tpsum, x, xc, wq, wk, wv16, out)
    return nc


def _body(nc, tc, const, mpool, apsum, tpsum, x, xc, wq, wk, wv16, out):
    # ---- DMAs: sig_rep half 0 gates the first masks — it goes first -----
    sig_rep = const.tile([P, S], f16)
    nc.sync.dma_start(sig_rep[:, 0:HALF], x[0:1, 0:HALF].to_broadcast([P, HALF]))
    colq = const.tile([P, 2 * NJ], f16)
    nc.scalar.dma_start(colq[:], xc)
    nc.sync.dma_start(sig_rep[:, HALF:S], x[0:1, HALF:S].to_broadcast([P, HALF]))
    wq_t = const.tile([P, D], f32)
    wk_t = const.tile([P, D], f32)
    nc.sync.dma_start(wq_t[:], wq.to_broadcast([P, D]))
    nc.sync.dma_start(wk_t[:], wk.to_broadcast([P, D]))
    wv_rep = const.tile([P, D], f16)
    nc.sync.dma_start(wv_rep[:], wv16.to_broadcast([P, D]))

    # ---- ACT: act-table preload during the DMA window -------------------
    dummy = const.tile([1, 1], f32)
    nc.vector.memset(dummy[:], 0.0)
    nc.scalar.activation(dummy[:], dummy[:], Act.Sign)

    # ---- DVE prologue: columns then straight into masks -----------------
    ones = const.tile([P, P], f16)
    nc.vector.memset(ones[:], 1.0)
    colf = const.tile([P, 2 * NJ], f32)
    nc.vector.tensor_copy(colf[:], colq[:])
    sig_col = colf[:, 0:NJ]
    h_col = colf[:, NJ : 2 * NJ]
    negsig = const.tile([P, NJ], f32)
    nc.vector.tensor_scalar_mul(negsig[:], sig_col, -1.0)

    gp = const.tile([P, NJ], f32)
    gm = const.tile([P, NJ], f32)
    ecol = const.tile([P, NJ], f32)
    nc.scalar.activation(gp[:], sig_col, Act.Exp, scale=0.5)
    nc.scalar.activation(gm[:], sig_col, Act.Exp, scale=-0.5)
    nc.scalar.activation(ecol[:], sig_col, Act.Exp, scale=1.0)

    # ---- Pool prologue: moments, c-chain --------------------------------
    mom = const.tile([P, 8 * NJ], f16)
    nc.gpsimd.memset(mom[:], 0.0)
    h2 = const.tile([P, NJ], f32)
    nc.gpsimd.tensor_mul(h2[:], h_col, h_col)
    momv = mom[:].rearrange("p (j m) -> p j m", m=8)
    nc.gpsimd.tensor_copy(momv[:, :, 0], gp[:])
    nc.gpsimd.tensor_mul(momv[:, :, 1], gp[:], h_col)
    nc.gpsimd.tensor_mul(momv[:, :, 2], gp[:], h2[:])
    nc.gpsimd.tensor_copy(momv[:, :, 4], gm[:])
    nc.gpsimd.tensor_mul(momv[:, :, 5], gm[:], h_col)
    nc.gpsimd.tensor_mul(momv[:, :, 6], gm[:], h2[:])
    momh3 = const.tile([P, 8 * len(ACT_JS)], f16)
    nc.gpsimd.tensor_scalar_mul(momh3[:], mom[:, 8 * ACT_JS[0] : 8 * (ACT_JS[-1] + 1)], 0.5)

    wqk = const.tile([P, D], f32)
    nc.gpsimd.tensor_mul(wqk[:], wq_t[:], wk_t[:])
    h16 = const.tile([P, NJ], f32)  # h/16 so a = (h/16) * (Wq.Wk)
    nc.gpsimd.tensor_scalar_mul(h16[:], h_col, 1.0 / 16.0)

    # ---- PE: totals -----------------------------------------------------
    psum_t = tpsum.tile([P, 7], f32, tag="pt")
    for j in range(NJ):
        nc.tensor.matmul(
            psum_t[:],
            ones[:],
            mom[:, 8 * j : 8 * j + 7],
            start=(j == 0),
            stop=(j == NJ - 1),
            skip_group_check=True,
        )

    # ---- Pool: per-query globals (t2/c_red filled by DVE post-h0-masks) -
    t2 = const.tile([P, 7], f32)  # T_m / 2
    c_red = const.tile([P, 1], f32)
    a_col = const.tile([P, NJ], f32)
    su2 = const.tile([P, NJ], f32)  # (T4 + a*T5)/2
    sup2 = const.tile([P, NJ], f32)  # (T5 + a*T6)/2
    g1 = const.tile([P, NJ], f32)  # (T0 + a*T1)/2
    g2 = const.tile([P, NJ], f32)  # (T1 + a*T2)/2
    esu = const.tile([P, NJ], f32)
    esup = const.tile([P, NJ], f32)

    def pool_globals():
        nc.gpsimd.tensor_scalar_mul(a_col[:], h16[:], c_red[:])
        nc.gpsimd.tensor_scalar(su2[:], a_col[:], t2[:, 5:6], t2[:, 4:5], op0=Alu.mult, op1=Alu.add)
        nc.gpsimd.tensor_scalar(sup2[:], a_col[:], t2[:, 6:7], t2[:, 5:6], op0=Alu.mult, op1=Alu.add)
        nc.gpsimd.tensor_scalar(g1[:], a_col[:], t2[:, 1:2], t2[:, 0:1], op0=Alu.mult, op1=Alu.add)
        nc.gpsimd.tensor_scalar(g2[:], a_col[:], t2[:, 2:3], t2[:, 1:2], op0=Alu.mult, op1=Alu.add)
        nc.gpsimd.tensor_mul(esu[:], ecol[:], su2[:])
        nc.gpsimd.tensor_add(esu[:], esu[:], g1[:])
        nc.gpsimd.tensor_mul(esup[:], ecol[:], sup2[:])
        nc.gpsimd.tensor_add(esup[:], esup[:], g2[:])

    # ---- masks and matmuls ----------------------------------------------
    sgn = {}
    for j in ACT_JS:
        sgn[j] = const.tile([P, S], f16, name=f"sgn{j}", tag=f"sgn{j}")
    psum_a = {}
    mstate = {}
    for h in range(2):
        psum_a[h] = apsum.tile([P, 64], f32, tag=f"pa{h}", name=f"pa{h}")
        mstate[h] = {"first": True}

    def act_sgn_half(h):
        lo, hi = HALF * h, HALF * (h + 1)
        for j in ACT_JS:
            nc.scalar.activation(
                sgn[j][:, lo:hi], sig_rep[:, lo:hi], Act.Sign, bias=negsig[:, j : j + 1]
            )

    def dve_mask(h, j, eng=None):
        lo = HALF * h
        m = mpool.tile([P, HALF], f16, tag="mask")
        (eng or nc.vector).tensor_scalar(
            m[:],
            sig_rep[:, lo : lo + HALF],
            sig_col[:, j : j + 1],
            0.5,
            op0=Alu.is_ge,
            op1=Alu.subtract,
        )
        st = mstate[h]
        for il in range(8):
            nc.tensor.matmul(
                psum_a[h][:, 8 * il : 8 * il + 8],
                m[:, P * il : P * (il + 1)],
                mom[:, 8 * j : 8 * j + 8],
                start=st["first"],
                stop=False,
                skip_group_check=True,
            )
            st["first"] = False

    def act_matmuls(h):
        lo = HALF * h
        for jx, j in enumerate(ACT_JS):
            for il in range(8):
                last = jx == len(ACT_JS) - 1 and il == 7
                nc.tensor.matmul(
                    psum_a[h][:, 8 * il : 8 * il + 8],
                    sgn[j][:, lo + P * il : lo + P * (il + 1)],
                    momh3[:, 8 * jx : 8 * jx + 8],
                    start=False,
                    stop=last,
                    skip_group_check=True,
                )

    out_r = out.rearrange("(i p) d -> p i d", p=P)
    comb = {}

    def acopy(h):
        acp = const.tile([P, 64], f32, name=f"acp{h}", tag=f"acp{h}")
        nc.scalar.copy(acp[:], psum_a[h][:])
        comb[h] = {"acp": acp}

    def pool_combine(h):
        acp = comb[h]["acp"]
        A = acp[:].rearrange("p (i m) -> p m i", m=8)
        cs = slice(8 * h, 8 * (h + 1))
        eh = ecol[:, cs]

        def ptt(name, in0, in1, op):
            t = const.tile([P, 8], f32, name=name + str(h), tag=name + str(h))
            nc.gpsimd.tensor_tensor(t[:], in0, in1, op=op)
            return t

        w0 = ptt("w0", eh, A[:, 4], Alu.mult)
        w1 = ptt("w1", eh, A[:, 5], Alu.mult)
        w2 = ptt("w2", eh, A[:, 6], Alu.mult)
        comb[h]["q0"] = ptt("q0", A[:, 0], w0[:], Alu.subtract)
        comb[h]["q1"] = ptt("q1", A[:, 1], w1[:], Alu.subtract)
        comb[h]["q2"] = ptt("q2", A[:, 2], w2[:], Alu.subtract)

    def dve_combine(h):
        cs = slice(8 * h, 8 * (h + 1))
        ah = a_col[:, cs]
        q0, q1, q2 = comb[h]["q0"], comb[h]["q1"], comb[h]["q2"]

        def vtt(name, in0, in1, op):
            t = const.tile([P, 8], f32, name=name + str(h), tag=name + str(h))
            nc.vector.tensor_tensor(t[:], in0, in1, op=op)
            return t

        u1 = vtt("u1", ah, q1[:], Alu.mult)
        d1 = vtt("d1", q0[:], u1[:], Alu.add)
        den = vtt("dn", d1[:], esu[:, cs], Alu.add)
        z1 = vtt("z1", ah, q2[:], Alu.mult)
        n1 = vtt("n1", q1[:], z1[:], Alu.add)
        num = vtt("nm", n1[:], esup[:, cs], Alu.add)
        inv = const.tile([P, 8], f32, name=f"inv{h}", tag=f"inv{h}")
        nc.vector.reciprocal_approx_fast(inv[:], den[:])
        comb[h]["aout"] = vtt("ao", num[:], inv[:], Alu.mult)

    def outers_quarter(qq, engines):
        h = qq // 2
        q = qq % 2
        aout = comb[h]["aout"]
        ob = const.tile([P, 4 * D], f16, name=f"ob{qq}", tag=f"ob{qq}")
        for il4 in range(4):
            sc = aout[:, 4 * q + il4 : 4 * q + il4 + 1]
            dst = ob[:, D * il4 : D * (il4 + 1)]
            engines[il4].tensor_scalar_mul(dst, wv_rep[:], sc)
        return ob

    def out_dma(qq, ob):
        nc.sync.dma_start(
            out_r[:, 4 * qq : 4 * (qq + 1)],
            ob[:].rearrange("p (i d) -> p i d", d=D),
        )

    OE = [nc.vector, nc.vector, nc.gpsimd, nc.gpsimd]

    # ---- emission: h0 masks ---------------------------------------------
    act_sgn_half(0)  # ACT: sgn h0 pieces (after exps)
    for j in DVE_JS:
        dve_mask(0, j)
    act_matmuls(0)  # PE: ACT-mask matmuls h0 (incl stop)
    # ACT idle window before the h0 psum closes: totals/2 and c reduction
    nc.scalar.mul(t2[:], psum_t[:], 0.5)
    wqks = const.tile([P, D], f32)
    nc.scalar.activation(wqks[:], wqk[:], Act.Copy, accum_out=c_red[:])
    acopy(0)  # ACT: psum->sbuf for h0 (before sgn h1 in ACT order)
    act_sgn_half(1)  # ACT: sgn h1 pieces
    pool_globals()  # Pool: a, su2/sup2/g1/g2, esu/esup (needs t2, c_red)
    pool_combine(0)  # Pool: w/q for h0

    for j in POOL_H1_JS:
        dve_mask(1, j, eng=nc.gpsimd)

    # ---- h1 masks part 1, then h0 combine + first outputs ---------------
    for j in DVE_H1_JS[:5]:
        dve_mask(1, j)
    dve_combine(0)
    out_dma(0, outers_quarter(0, OE))
    out_dma(1, outers_quarter(1, OE))

    # ---- h1 masks part 2, close h1, outputs -----------------------------
    for j in DVE_H1_JS[5:]:
        dve_mask(1, j)
    act_matmuls(1)
    acopy(1)  # ACT
    pool_combine(1)  # Pool
    dve_combine(1)
    OEV = [nc.vector] * 4
    out_dma(2, outers_quarter(2, OEV))
    ob3 = outers_quarter(3, OEV)
    nc.sync.dma_start(out_r[:, 12:14], ob3[:, 0 : 2 * D].rearrange("p (i d) -> p i d", d=D))
    nc.scalar.dma_start(out_r[:, 14:16], ob3[:, 2 * D : 4 * D].rearrange("p (i d) -> p i d", d=D))


_NC = {}


def _get_nc():
    if "nc" not in _NC:
        nc = bacc.Bacc("TRN2", target_bir_lowering=False, debug=False, num_devices=N_CORES)
        build_kernel(nc)
        nc.compile()
        _NC["nc"] = nc
    return _NC["nc"]


def kernel(inputs: np.ndarray, Wq: np.ndarray, Wk: np.ndarray, Wv: np.ndarray) -> np.ndarray:
    assert inputs.shape == (N_CORES, S, 2), inputs.shape
    nc = _get_nc()
    wq = np.ascontiguousarray(Wq, dtype=np.float32)
    wk = np.ascontiguousarray(Wk, dtype=np.float32)
    wv16 = np.ascontiguousarray(np.asarray(Wv, dtype=np.float32).astype(np.float16))
    xs = [
        np.ascontiguousarray(np.asarray(inputs[b], dtype=np.float32).T.astype(np.float16))
        for b in range(N_CORES)
    ]
    # xc[p, c*NJ + j] = x[c, j*128 + p]
    xcs = [
        np.ascontiguousarray(xb.reshape(2, NJ, P).transpose(2, 0, 1).reshape(P, 2 * NJ))
        for xb in xs
    ]
    in_maps = [
        {"x": xs[b], "xc": xcs[b], "wq": wq, "wk": wk, "wv16": wv16}
        for b in range(N_CORES)
    ]
    res = run_bass_kernel_spmd(nc, in_maps, core_ids=list(range(N_CORES)))
    return np.stack([r["out"].astype(np.float32) for r in res.results], axis=0)


# revision 20
# speedup vs baseline: 2.8237x; 1.0035x over previous
"""Distance-weighted self-attention on 8 Trainium2 NeuronCores.

The reference network is rank-1 in d_model:
  q = h*Wq, k = h*Wk, v = h*Wv  (h = heights column)
  logits[p,k] = c*h_p*h_k - 0.5*|sig_p - sig_k|,  c = (Wq.Wk)/sqrt(256)
  out[p,:]   = (sum_k softmax(logits)[p,k]*h_k) * Wv.

Key identity used here: with L(p) = {k : sig_k <= sig_p},
  exp(-0.5|sig_p - sig_k|) = e^{-0.5 sig_p} e^{+0.5 sig_k}   for k in L(p)
                           = e^{+0.5 sig_p} e^{-0.5 sig_k}   otherwise,
and since |c*h_p*h_k| <~ 0.05, exp(c h_p h_k) = 1 + c h_p h_k to ~1e-3.
With the +-1/2 step convention s(p,k) = [sig_k <= sig_p] - 1/2 (ties -> 0,
exact because both branch formulas agree at sig_k == sig_p), the masked
sums A_m(p) = sum_k s(p,k) g_k h_k^m obey A_m = P_m + T_m/2 with P from
the sign-mask matmuls and T the plain totals.  Dividing num/den by
e^{-0.5 sig_p} (cancels in the ratio):
  den'_p = Q0 + a*Q1 + E*su2 + g1,  Q_m = P_m - E*P_{m+4}, E = e^{sig_p}
  num'_p = Q1 + a*Q2 + E*sup2 + g2
so the only O(S^2) device work is one 4x-mode DVE compare per key-chunk
half (3 chunks use +-1 Sign masks on the scalar engine) and tiny PE
matmuls lhsT=mask[128,128] x rhs=moments[128,8] accumulated into a
[128,8]-per-query-chunk PSUM bank (matmul start=True resets a whole PSUM
bank, so each half owns one bank and only its first matmul sets start).

The output outer product a x Wv runs as per-partition-scalar multiplies
into fp16, written back per quarter (host upcasts to f32), so the 1 MB
writeback overlaps the second half's mask phase.  Engines execute their
streams strictly in order, so emission order is laid out explicitly: the
scalar engine interleaves its PSUM->SBUF copy between the two sign-mask
halves, and the vector engine interleaves the half-0 combine into the
half-1 mask stream.
"""

import os
import sys

import numpy as np

for _p in ("/opt/trn_rl_repo", "/root/.axon_site/_ro/trn_rl_repo"):
    if os.path.isdir(_p) and _p not in sys.path:
        sys.path.append(_p)

import concourse.bacc as bacc
import concourse.bass as bass
import concourse.mybir as mybir
import concourse.tile as tile
from concourse.bass_utils import run_bass_kernel_spmd

S = 2048
D = 256
P = 128
NJ = S // P  # 16
N_CORES = 8
HALF = S // 2

f32 = mybir.dt.float32
f16 = mybir.dt.float16
Alu = mybir.AluOpType
Act = mybir.ActivationFunctionType

ACT_JS = (13, 14, 15)  # key chunks whose masks run on the scalar engine
POOL_H1_JS = (11, 12)  # key chunks whose half-1 masks run on gpsimd
DVE_JS = tuple(j for j in range(NJ) if j not in ACT_JS)
DVE_H1_JS = tuple(j for j in DVE_JS if j not in POOL_H1_JS)


def build_kernel(nc: bass.Bass):
    x = nc.dram_tensor("x", [2, S], f16, kind="ExternalInput").ap()
    # host-pretransposed columns: xc[p, c*NJ + j] = x[c, j*128 + p]
    xc = nc.dram_tensor("xc", [P, 2 * NJ], f16, kind="ExternalInput").ap()
    # Wq/Wk in column layout [128, 2] (host-reshaped) for the PE c-reduction
    wq = nc.dram_tensor("wq", [P, 2], f32, kind="ExternalInput").ap()
    wk = nc.dram_tensor("wk", [P, 2], f32, kind="ExternalInput").ap()
    wv16 = nc.dram_tensor("wv16", [1, D], f16, kind="ExternalInput").ap()
    out = nc.dram_tensor("out", [S, D], f16, kind="ExternalOutput").ap()

    with tile.TileContext(nc) as tc:
        from contextlib import ExitStack

        with ExitStack() as ctx:
            const = ctx.enter_context(tc.tile_pool(name="const", bufs=1))
            mpool = ctx.enter_context(tc.tile_pool(name="mpool", bufs=8))
            apsum = ctx.enter_context(
                tc.tile_pool(name="apsum", bufs=1, space=bass.MemorySpace.PSUM)
            )
            tpsum = ctx.enter_context(
                tc.tile_pool(name="tpsum", bufs=1, space=bass.MemorySpace.PSUM)
            )
            _body(nc, tc, const, mpool, apsum, tpsum, x, xc, wq, wk, wv16, out)
    return nc


def _body(nc, tc, const, mpool, apsum, tpsum, x, xc, wq, wk, wv16, out):
    # ---- DMAs: sig_rep half 0 gates the first masks — it goes first -----
    sig_rep = const.tile([P, S], f16)
    nc.sync.dma_start(sig_rep[:, 0:HALF], x[0:1, 0:HALF].to_broadcast([P, HALF]))
    colq = const.tile([P, 2 * NJ], f16)
    nc.scalar.dma_start(colq[:], xc)
    nc.sync.dma_start(sig_rep[:, HALF:S], x[0:1, HALF:S].to_broadcast([P, HALF]))
    wq_t = const.tile([P, 2], f32)
    wk_t = const.tile([P, 2], f32)
    nc.sync.dma_start(wq_t[:], wq)
    nc.sync.dma_start(wk_t[:], wk)
    wv_rep = const.tile([P, D], f16)
    nc.sync.dma_start(wv_rep[:], wv16.to_broadcast([P, D]))

    # ---- ACT: act-table preload during the DMA window -------------------
    dummy = const.tile([1, 1], f32)
    nc.vector.memset(dummy[:], 0.0)
    nc.scalar.activation(dummy[:], dummy[:], Act.Sign)

    # ---- DVE prologue: columns then straight into masks -----------------
    ones = const.tile([P, P], f16)
    nc.vector.memset(ones[:], 1.0)
    colf = const.tile([P, 2 * NJ], f32)
    nc.vector.tensor_copy(colf[:], colq[:])
    sig_col = colf[:, 0:NJ]
    h_col = colf[:, NJ : 2 * NJ]
    negsig = const.tile([P, NJ], f32)
    nc.vector.tensor_scalar_mul(negsig[:], sig_col, -1.0)

    gp = const.tile([P, NJ], f32)
    gm = const.tile([P, NJ], f32)
    ecol = const.tile([P, NJ], f32)
    nc.scalar.activation(gp[:], sig_col, Act.Exp, scale=0.5)
    nc.scalar.activation(gm[:], sig_col, Act.Exp, scale=-0.5)
    nc.scalar.activation(ecol[:], sig_col, Act.Exp, scale=1.0)

    # ---- Pool prologue: moments, c-chain --------------------------------
    mom = const.tile([P, 8 * NJ], f16)
    nc.gpsimd.memset(mom[:], 0.0)
    h2 = const.tile([P, NJ], f32)
    nc.gpsimd.tensor_mul(h2[:], h_col, h_col)
    momv = mom[:].rearrange("p (j m) -> p j m", m=8)
    nc.gpsimd.tensor_copy(momv[:, :, 0], gp[:])
    nc.gpsimd.tensor_mul(momv[:, :, 1], gp[:], h_col)
    nc.gpsimd.tensor_mul(momv[:, :, 2], gp[:], h2[:])
    nc.gpsimd.tensor_copy(momv[:, :, 4], gm[:])
    nc.gpsimd.tensor_mul(momv[:, :, 5], gm[:], h_col)
    nc.gpsimd.tensor_mul(momv[:, :, 6], gm[:], h2[:])
    momh3 = const.tile([P, 8 * len(ACT_JS)], f16)
    nc.gpsimd.tensor_scalar_mul(momh3[:], mom[:, 8 * ACT_JS[0] : 8 * (ACT_JS[-1] + 1)], 0.5)

    wqkc = const.tile([P, 2], f16)
    nc.gpsimd.tensor_mul(wqkc[:], wq_t[:], wk_t[:])
    h16 = const.tile([P, NJ], f32)  # h/16 so a = (h/16) * (Wq.Wk)
    nc.gpsimd.tensor_scalar_mul(h16[:], h_col, 1.0 / 16.0)

    # ---- PE: c-reduction then totals ------------------------------------
    psum_c = tpsum.tile([P, 2], f32, tag="pc")
    nc.tensor.matmul(psum_c[:], ones[:], wqkc[:], start=True, stop=True, skip_group_check=True)
    psum_t = tpsum.tile([P, 7], f32, tag="pt")
    for j in range(NJ):
        nc.tensor.matmul(
            psum_t[:],
            ones[:],
            mom[:, 8 * j : 8 * j + 7],
            start=(j == 0),
            stop=(j == NJ - 1),
            skip_group_check=True,
        )

    # ---- Pool: per-query globals (t2/c_red filled by DVE post-h0-masks) -
    t2 = const.tile([P, 7], f32)  # T_m / 2
    c_col = const.tile([P, 1], f32)
    a_col = const.tile([P, NJ], f32)
    su2 = const.tile([P, NJ], f32)  # (T4 + a*T5)/2
    sup2 = const.tile([P, NJ], f32)  # (T5 + a*T6)/2
    g1 = const.tile([P, NJ], f32)  # (T0 + a*T1)/2
    g2 = const.tile([P, NJ], f32)  # (T1 + a*T2)/2
    esu = const.tile([P, NJ], f32)
    esup = const.tile([P, NJ], f32)

    def pool_globals():
        nc.gpsimd.tensor_scalar_mul(a_col[:], h16[:], c_col[:])
        nc.gpsimd.tensor_scalar(su2[:], a_col[:], t2[:, 5:6], t2[:, 4:5], op0=Alu.mult, op1=Alu.add)
        nc.gpsimd.tensor_scalar(sup2[:], a_col[:], t2[:, 6:7], t2[:, 5:6], op0=Alu.mult, op1=Alu.add)
        nc.gpsimd.tensor_scalar(g1[:], a_col[:], t2[:, 1:2], t2[:, 0:1], op0=Alu.mult, op1=Alu.add)
        nc.gpsimd.tensor_scalar(g2[:], a_col[:], t2[:, 2:3], t2[:, 1:2], op0=Alu.mult, op1=Alu.add)
        nc.gpsimd.tensor_mul(esu[:], ecol[:], su2[:])
        nc.gpsimd.tensor_add(esu[:], esu[:], g1[:])
        nc.gpsimd.tensor_mul(esup[:], ecol[:], sup2[:])
        nc.gpsimd.tensor_add(esup[:], esup[:], g2[:])

    # ---- masks and matmuls ----------------------------------------------
    sgn = {}
    for j in ACT_JS:
        sgn[j] = const.tile([P, S], f16, name=f"sgn{j}", tag=f"sgn{j}")
    psum_a = {}
    mstate = {}
    for h in range(2):
        psum_a[h] = apsum.tile([P, 64], f32, tag=f"pa{h}", name=f"pa{h}")
        mstate[h] = {"first": True}

    def act_sgn_half(h):
        lo, hi = HALF * h, HALF * (h + 1)
        for j in ACT_JS:
            nc.scalar.activation(
                sgn[j][:, lo:hi], sig_rep[:, lo:hi], Act.Sign, bias=negsig[:, j : j + 1]
            )

    def dve_mask(h, j, eng=None):
        lo = HALF * h
        m = mpool.tile([P, HALF], f16, tag="mask")
        (eng or nc.vector).tensor_scalar(
            m[:],
            sig_rep[:, lo : lo + HALF],
            sig_col[:, j : j + 1],
            0.5,
            op0=Alu.is_ge,
            op1=Alu.subtract,
        )
        st = mstate[h]
        for il in range(8):
            nc.tensor.matmul(
                psum_a[h][:, 8 * il : 8 * il + 8],
                m[:, P * il : P * (il + 1)],
                mom[:, 8 * j : 8 * j + 8],
                start=st["first"],
                stop=False,
                skip_group_check=True,
            )
            st["first"] = False

    def act_matmuls(h):
        lo = HALF * h
        for jx, j in enumerate(ACT_JS):
            for il in range(8):
                last = jx == len(ACT_JS) - 1 and il == 7
                nc.tensor.matmul(
                    psum_a[h][:, 8 * il : 8 * il + 8],
                    sgn[j][:, lo + P * il : lo + P * (il + 1)],
                    momh3[:, 8 * jx : 8 * jx + 8],
                    start=False,
                    stop=last,
                    skip_group_check=True,
                )

    out_r = out.rearrange("(i p) d -> p i d", p=P)
    comb = {}

    def acopy(h):
        acp = const.tile([P, 64], f32, name=f"acp{h}", tag=f"acp{h}")
        nc.scalar.copy(acp[:], psum_a[h][:])
        comb[h] = {"acp": acp}

    def pool_combine(h):
        acp = comb[h]["acp"]
        A = acp[:].rearrange("p (i m) -> p m i", m=8)
        cs = slice(8 * h, 8 * (h + 1))
        eh = ecol[:, cs]

        def ptt(name, in0, in1, op):
            t = const.tile([P, 8], f32, name=name + str(h), tag=name + str(h))
            nc.gpsimd.tensor_tensor(t[:], in0, in1, op=op)
            return t

        w0 = ptt("w0", eh, A[:, 4], Alu.mult)
        w1 = ptt("w1", eh, A[:, 5], Alu.mult)
        w2 = ptt("w2", eh, A[:, 6], Alu.mult)
        comb[h]["q0"] = ptt("q0", A[:, 0], w0[:], Alu.subtract)
        comb[h]["q1"] = ptt("q1", A[:, 1], w1[:], Alu.subtract)
        comb[h]["q2"] = ptt("q2", A[:, 2], w2[:], Alu.subtract)

    def dve_combine(h):
        cs = slice(8 * h, 8 * (h + 1))
        ah = a_col[:, cs]
        q0, q1, q2 = comb[h]["q0"], comb[h]["q1"], comb[h]["q2"]

        def vtt(name, in0, in1, op):
            t = const.tile([P, 8], f32, name=name + str(h), tag=name + str(h))
            nc.vector.tensor_tensor(t[:], in0, in1, op=op)
            return t

        u1 = vtt("u1", ah, q1[:], Alu.mult)
        d1 = vtt("d1", q0[:], u1[:], Alu.add)
        den = vtt("dn", d1[:], esu[:, cs], Alu.add)
        z1 = vtt("z1", ah, q2[:], Alu.mult)
        n1 = vtt("n1", q1[:], z1[:], Alu.add)
        num = vtt("nm", n1[:], esup[:, cs], Alu.add)
        inv = const.tile([P, 8], f32, name=f"inv{h}", tag=f"inv{h}")
        nc.vector.reciprocal_approx_fast(inv[:], den[:])
        comb[h]["aout"] = vtt("ao", num[:], inv[:], Alu.mult)

    def outers_quarter(qq, engines):
        h = qq // 2
        q = qq % 2
        aout = comb[h]["aout"]
        ob = const.tile([P, 4 * D], f16, name=f"ob{qq}", tag=f"ob{qq}")
        for il4 in range(4):
            sc = aout[:, 4 * q + il4 : 4 * q + il4 + 1]
            dst = ob[:, D * il4 : D * (il4 + 1)]
            engines[il4].tensor_scalar_mul(dst, wv_rep[:], sc)
        return ob

    def out_dma(qq, ob):
        nc.sync.dma_start(
            out_r[:, 4 * qq : 4 * (qq + 1)],
            ob[:].rearrange("p (i d) -> p i d", d=D),
        )

    OE = [nc.vector, nc.vector, nc.gpsimd, nc.gpsimd]

    OEV = [nc.vector] * 4

    # ---- emission ---------------------------------------------------------
    act_sgn_half(0)  # ACT: sgn h0 pieces (after exps)
    # DVE: h0 masks j0-j2, then the tiny c combine (psum_c ready ~4.5)
    for j in DVE_JS[:3]:
        dve_mask(0, j)
    nc.vector.tensor_scalar(c_col[:], psum_c[:, 0:1], psum_c[:, 1:2], None, op0=Alu.add)
    for j in DVE_JS[3:]:
        dve_mask(0, j)
    act_matmuls(0)  # PE: ACT-mask matmuls h0 (incl stop)
    nc.scalar.mul(t2[:], psum_t[:], 0.5)  # ACT idle window
    acopy(0)  # ACT: psum->sbuf for h0 (before sgn h1 in ACT order)
    act_sgn_half(1)  # ACT: sgn h1 pieces

    # Pool: its two h1 masks, then globals and h0 w/q
    for j in POOL_H1_JS:
        dve_mask(1, j, eng=nc.gpsimd)
    pool_globals()
    pool_combine(0)

    # DVE: uninterrupted h1 masks, then combine h0 + all early outputs
    for j in DVE_H1_JS:
        dve_mask(1, j)
    act_matmuls(1)
    dve_combine(0)
    out_dma(0, outers_quarter(0, OEV))
    out_dma(1, outers_quarter(1, OEV))

    acopy(1)  # ACT (after sgn h1 in its stream)
    pool_combine(1)  # Pool w/q h1
    dve_combine(1)
    out_dma(2, outers_quarter(2, OEV))
    ob3 = outers_quarter(3, OEV)
    nc.sync.dma_start(out_r[:, 12:14], ob3[:, 0 : 2 * D].rearrange("p (i d) -> p i d", d=D))
    nc.scalar.dma_start(out_r[:, 14:16], ob3[:, 2 * D : 4 * D].rearrange("p (i d) -> p i d", d=D))


_NC = {}


def _get_nc():
    if "nc" not in _NC:
        nc = bacc.Bacc("TRN2", target_bir_lowering=False, debug=False, num_devices=N_CORES)
        build_kernel(nc)
        nc.compile()
        _NC["nc"] = nc
    return _NC["nc"]


def kernel(inputs: np.ndarray, Wq: np.ndarray, Wk: np.ndarray, Wv: np.ndarray) -> np.ndarray:
    assert inputs.shape == (N_CORES, S, 2), inputs.shape
    nc = _get_nc()
    wq = np.ascontiguousarray(np.asarray(Wq, np.float32).reshape(2, P).T)
    wk = np.ascontiguousarray(np.asarray(Wk, np.float32).reshape(2, P).T)
    wv16 = np.ascontiguousarray(np.asarray(Wv, dtype=np.float32).astype(np.float16))
    xs = [
        np.ascontiguousarray(np.asarray(inputs[b], dtype=np.float32).T.astype(np.float16))
        for b in range(N_CORES)
    ]
    # xc[p, c*NJ + j] = x[c, j*128 + p]
    xcs = [
        np.ascontiguousarray(xb.reshape(2, NJ, P).transpose(2, 0, 1).reshape(P, 2 * NJ))
        for xb in xs
    ]
    in_maps = [
        {"x": xs[b], "xc": xcs[b], "wq": wq, "wk": wk, "wv16": wv16}
        for b in range(N_CORES)
    ]
    res = run_bass_kernel_spmd(nc, in_maps, core_ids=list(range(N_CORES)))
    return np.stack([r["out"].astype(np.float32) for r in res.results], axis=0)
